# revision 23
# baseline (speedup 1.0000x reference)
"""MPNN layer on 8 Trainium2 NeuronCores (Bass/Tile) - v4, gather-free.

Math (reference):
    m_edge = relu(x[dst] @ W1a^T + x[src] @ W1b^T + h @ W1c^T)        [E, D]
    m_node = segment_sum(m_edge, dst, N)                               [N, D]
    y      = m_node @ W2^T                                             [N, D]
    out_e  = relu(LN(snorm_n_e * y[src_e]))                            [E, D]

LN decomposition (exact, s>0):
    LN(s*v)*gamma+beta = (v - mu_v)*gamma * a_e + beta,
    a_e = s*rsqrt(s^2*var_v + eps) = rsqrt(var_v + eps/s^2)
and for beta==0:  relu(yc*gamma * a_e) = a_e * relu(yc*gamma)  (a_e > 0),
so the per-node record stores relu'd centered y and var; the per-edge part is
a scalar multiply.

Sharding (no collectives, no device gathers; each core independent):
  phase 1: edges bucketed by dst-owner core + 128-node dst block; h and
    x[src] are laid out per-edge by the host (input permutation / gather on
    host). Per-edge terms via per-tile matmuls; x[dst] selected on PE via a
    one-hot built from broadcast+is_equal; segment-sum via one-hot matmul.
  phase 1.5: per block: y = m@W2, mean/var/relu-center; records stay in SBUF.
  phase 2: edges bucketed by src-owner core + src block (records are local).
    One-hot select of record + var on PE; a = 1/sqrt((varx+c2x)/128) on
    column vectors; per-edge scale fused into the psum->sbuf copy. Phase-2
    superchunks are interleaved into the block loop as their blocks complete.
  Output written bf16 in bucketed order; host inverts the permutation.
"""

import numpy as np
import ml_dtypes

from concourse import bacc, tile, mybir
from concourse.bass_utils import run_bass_kernel_spmd

P = 128
LN_EPS = 1e-5
BF16 = ml_dtypes.bfloat16

# ----------------------------------------------------------------------------
# host-side planning
# ----------------------------------------------------------------------------


def _ceil_to(x, m):
    return -(-x // m) * m


class Plan:
    def __init__(self, n_nodes, n_edges, src, dst, nc=8):
        self.nc = nc
        self.n_nodes = n_nodes
        self.n_edges = n_edges
        self.npc = n_nodes // nc
        assert self.npc * nc == n_nodes
        self.npc_pad = _ceil_to(self.npc, P)
        self.nblk = self.npc_pad // P

        src = np.asarray(src).astype(np.int64)
        dst = np.asarray(dst).astype(np.int64)
        self.src, self.dst = src, dst

        # ---- phase 1: bucket edges by (dst core, dst block)
        core1 = dst // self.npc
        blk1 = (dst - core1 * self.npc) // P
        key1 = core1 * self.nblk + blk1
        t1max = 1
        self.p1 = []           # [core][block] -> ids
        for c in range(nc):
            blocks = []
            for b in range(self.nblk):
                ids = np.nonzero(key1 == c * self.nblk + b)[0]
                blocks.append(ids)
                t1max = max(t1max, -(-len(ids) // P))
            self.p1.append(blocks)
        self.t_blk = _ceil_to(t1max, 4)         # tiles per block, mult of 4
        self.t1 = self.nblk * self.t_blk
        self.e1 = self.t1 * P

        # ---- phase 2: bucket edges by (src core, src block)
        core2 = src // self.npc
        blk2 = (src - core2 * self.npc) // P
        key2 = core2 * self.nblk + blk2
        t2b = 1
        self.p2 = []
        for c in range(nc):
            blocks = []
            for b in range(self.nblk):
                ids = np.nonzero(key2 == c * self.nblk + b)[0]
                blocks.append(ids)
                t2b = max(t2b, -(-len(ids) // P))
            self.p2.append(blocks)
        self.t2blk = t2b
        self.t2 = _ceil_to(self.nblk * self.t2blk, 32)
        self.n_sc = self.t2 // 32
        self.e2 = self.t2 * P

    # ---- per-core input arrays -------------------------------------------
    def core_inputs(self, c, x_bf, h, snorm_n):
        p = self
        f32 = np.float32

        slots = np.full(p.e1, -1, dtype=np.int64)
        for b, ids in enumerate(p.p1[c]):
            base = b * p.t_blk * P
            slots[base: base + len(ids)] = ids
        pad = slots < 0
        e_ids = np.where(pad, 0, slots)

        h_t = np.ascontiguousarray(h[e_ids].T).astype(BF16)
        h_t[:, pad] = BF16(0.0)
        xs_t = np.ascontiguousarray(x_bf[self.src[e_ids]].T)
        xs_t[:, pad] = BF16(0.0)

        dst_rel = (self.dst[e_ids] - c * p.npc -
                   (np.arange(p.e1) // (p.t_blk * P)) * P)
        dst_rel = np.where(pad, -1.0, dst_rel.astype(f32)).astype(f32)
        dstrel_col = dst_rel.reshape(p.t1, P).T.copy()        # [128, t1] f32
        dstrel_row = dst_rel.astype(BF16).reshape(1, p.e1)    # [1, e1] bf16

        # phase-2 slots
        slots2 = np.full(p.e2, -1, dtype=np.int64)
        for b, ids in enumerate(p.p2[c]):
            base = b * p.t2blk * P
            slots2[base: base + len(ids)] = ids
        pad2 = slots2 < 0
        e2_ids = np.where(pad2, 0, slots2)
        src_rel = self.src[e2_ids] - c * p.npc - \
            np.minimum(np.arange(p.e2) // (p.t2blk * P), p.nblk - 1) * P
        src_rel = np.where(pad2, -1.0, src_rel.astype(f32)).astype(f32)
        srcrel_row = src_rel.astype(BF16).reshape(1, p.e2)

        s = snorm_n.reshape(-1)[e2_ids].astype(np.float64)
        with np.errstate(divide="ignore", over="ignore"):
            c2x = P * LN_EPS / (s * s)          # 128 * eps / s^2  (inf ok)
        c2x = np.where(pad2, 1.0, c2x).astype(f32)
        c2c = c2x.reshape(p.t2, P).T.copy()                   # [128, t2]

        return {
            "h_t": h_t,
            "xs_t": xs_t,
            "dstrel_col": dstrel_col,
            "dstrel_row": dstrel_row,
            "srcrel_row": srcrel_row,
            "c2c": c2c,
            "x_tl": np.ascontiguousarray(np.pad(
                x_bf[c * p.npc: (c + 1) * p.npc],
                ((0, p.npc_pad - p.npc), (0, 0))).T),
        }, slots2


# ----------------------------------------------------------------------------
# bass program
# ----------------------------------------------------------------------------


def build_program(p: Plan, use_gamma, use_beta):
    dt = mybir.dt
    nc = bacc.Bacc(None)

    f32, bf16 = dt.float32, dt.bfloat16
    REC = 132          # rhs_blk row: [relu(yc*gamma)(128) | 128*var | pad]

    x_tl = nc.declare_dram_parameter("x_tl", [P, p.npc_pad], bf16, isOutput=False)
    h_t = nc.declare_dram_parameter("h_t", [P, p.e1], bf16, isOutput=False)
    xs_t = nc.declare_dram_parameter("xs_t", [P, p.e1], bf16, isOutput=False)
    w1aT = nc.declare_dram_parameter("w1aT", [P, P], bf16, isOutput=False)
    w1bT = nc.declare_dram_parameter("w1bT", [P, P], bf16, isOutput=False)
    w1cT = nc.declare_dram_parameter("w1cT", [P, P], bf16, isOutput=False)
    w2T = nc.declare_dram_parameter("w2T", [P, P], bf16, isOutput=False)
    dstrel_col_in = nc.declare_dram_parameter("dstrel_col", [P, p.t1], f32, isOutput=False)
    dstrel_row_in = nc.declare_dram_parameter("dstrel_row", [1, p.e1], bf16, isOutput=False)
    srcrel_row_in = nc.declare_dram_parameter("srcrel_row", [1, p.e2], bf16, isOutput=False)
    c2c_in = nc.declare_dram_parameter("c2c", [P, p.t2], f32, isOutput=False)
    ones_in = nc.declare_dram_parameter("ones_row", [1, P], bf16, isOutput=False)
    iota_in = nc.declare_dram_parameter("iota_col", [P, 1], f32, isOutput=False)
    iota_row_in = nc.declare_dram_parameter("iota_row", [P, P], bf16, isOutput=False)
    gamma_b = beta_b = None
    if use_gamma:
        gamma_b = nc.declare_dram_parameter("gamma_b", [P, P], f32, isOutput=False)
    if use_beta:
        beta_b = nc.declare_dram_parameter("beta_b", [P, P], f32, isOutput=False)

    out = nc.declare_dram_parameter("out", [p.e2, P], bf16, isOutput=True)

    with tile.TileContext(nc) as tc:
        with tc.tile_pool(name="const", bufs=1) as cpool, \
             tc.tile_pool(name="hx", bufs=2) as hpool, \
             tc.tile_pool(name="xsx", bufs=2) as xspool, \
             tc.tile_pool(name="row", bufs=2) as rpool, \
             tc.tile_pool(name="edge", bufs=3) as epool, \
             tc.tile_pool(name="blk", bufs=2) as bpool, \
             tc.tile_pool(name="oh2", bufs=9) as o2pool, \
             tc.tile_pool(name="p2s", bufs=2) as s2pool, \
             tc.tile_pool(name="outp", bufs=2) as opool, \
             tc.tile_pool(name="psA", bufs=2, space="PSUM") as psA, \
             tc.tile_pool(name="psB", bufs=2, space="PSUM") as psB, \
             tc.tile_pool(name="psC", bufs=2, space="PSUM") as psC:

            # ---- constants
            w1aT_sb = cpool.tile([P, P], bf16, tag="w1a")
            w1bT_sb = cpool.tile([P, P], bf16, tag="w1b")
            w1cT_sb = cpool.tile([P, P], bf16, tag="w1c")
            w2T_sb = cpool.tile([P, P], bf16, tag="w2")
            ones_sb = cpool.tile([1, P], bf16, tag="ones")
            iotac_sb = cpool.tile([P, 1], f32, tag="iotac")
            iotar_sb = cpool.tile([P, P], bf16, tag="iotar")
            dcol_sb = cpool.tile([P, p.t1], f32, tag="dcol")
            c2c_sb = cpool.tile([P, p.t2], f32, tag="c2c")
            nc.sync.dma_start(out=w1aT_sb[:], in_=w1aT[:])
            nc.sync.dma_start(out=w1bT_sb[:], in_=w1bT[:])
            nc.sync.dma_start(out=w1cT_sb[:], in_=w1cT[:])
            nc.sync.dma_start(out=w2T_sb[:], in_=w2T[:])
            nc.sync.dma_start(out=ones_sb[:], in_=ones_in[:])
            nc.sync.dma_start(out=iotac_sb[:], in_=iota_in[:])
            nc.sync.dma_start(out=iotar_sb[:], in_=iota_row_in[:])
            nc.sync.dma_start(out=dcol_sb[:], in_=dstrel_col_in[:])
            nc.sync.dma_start(out=c2c_sb[:], in_=c2c_in[:])
            gamma_sb = beta_sb = None
            if use_gamma:
                gamma_sb = cpool.tile([P, P], f32, tag="gam")
                nc.sync.dma_start(out=gamma_sb[:], in_=gamma_b[:])
            if use_beta:
                beta_sb = cpool.tile([P, P], f32, tag="bet")
                nc.sync.dma_start(out=beta_sb[:], in_=beta_b[:])

            # per-block records, persistent
            rhs_blk = [cpool.tile([P, REC], bf16, tag=f"rec{b}",
                                  name=f"rec{b}")
                       for b in range(p.nblk)]

            scratch = cpool.tile([P, P], f32, tag="scr")
            n4 = p.t_blk // 4

            # ---------------- phase 2 superchunk emitter ------------------
            def emit_superchunk(s):
                e0s = s * 32 * P
                sr_sb = rpool.tile([1, 32 * P], bf16, tag="srow")
                nc.sync.dma_start(out=sr_sb[:],
                                  in_=srcrel_row_in[:, e0s: e0s + 32 * P])

                psVc = psC.tile([P, 32], f32, tag="psxa")
                ohTs = []
                for g in range(8):
                    c0 = g * 4 * P
                    bc2 = psB.tile([P, 4 * P], f32, tag="bc")
                    nc.tensor.matmul(out=bc2[:], lhsT=ones_sb[:],
                                     rhs=sr_sb[:, c0:c0 + 4 * P],
                                     start=True, stop=True)
                    ohT2 = o2pool.tile([P, 4 * P], bf16, tag="ohT2")
                    nc.vector.tensor_scalar(
                        out=ohT2[:], in0=bc2[:], scalar1=iotac_sb[:],
                        scalar2=None, op0=mybir.AluOpType.is_equal)
                    ohTs.append(ohT2)
                    for tt in range(4):
                        t = s * 32 + g * 4 + tt
                        b = min(t // p.t2blk, p.nblk - 1)
                        nc.tensor.matmul(
                            out=psVc[:, g * 4 + tt: g * 4 + tt + 1],
                            lhsT=ohT2[:, tt * P:(tt + 1) * P],
                            rhs=rhs_blk[b][:, P:P + 1],
                            start=True, stop=True)
                vc = s2pool.tile([P, 32], f32, tag="vc")
                nc.vector.tensor_tensor(out=vc[:], in0=psVc[:],
                                        in1=c2c_sb[:, s * 32:(s + 1) * 32],
                                        op=mybir.AluOpType.add)
                rt = s2pool.tile([P, 32], f32, tag="rt")
                nc.scalar.activation(out=rt[:], in_=vc[:],
                                     func=mybir.ActivationFunctionType.Sqrt,
                                     scale=1.0 / P)
                a_sb = s2pool.tile([P, 32], f32, tag="a_sb")
                nc.vector.reciprocal(out=a_sb[:], in_=rt[:])

                out_sb = opool.tile([P, 32, P], bf16, tag="osb")
                for g in range(8):
                    ohT2 = ohTs[g]
                    sel4 = psA.tile([P, 4, P], f32, tag="m4")
                    for tt in range(4):
                        t = s * 32 + g * 4 + tt
                        b = min(t // p.t2blk, p.nblk - 1)
                        nc.tensor.matmul(out=sel4[:, tt, :],
                                         lhsT=ohT2[:, tt * P:(tt + 1) * P],
                                         rhs=rhs_blk[b][:, 0:P],
                                         start=True, stop=True)
                    for tt in range(4):
                        tsc = g * 4 + tt
                        if not use_beta:
                            if tt % 2 == 0:
                                nc.scalar.activation(
                                    out=out_sb[:, tsc, :], in_=sel4[:, tt, :],
                                    func=mybir.ActivationFunctionType.Copy,
                                    scale=a_sb[:, tsc:tsc + 1])
                            else:
                                nc.vector.tensor_scalar(
                                    out=out_sb[:, tsc, :], in0=sel4[:, tt, :],
                                    scalar1=a_sb[:, tsc:tsc + 1], scalar2=None,
                                    op0=mybir.AluOpType.mult)
                        else:
                            tmp = s2pool.tile([P, P], f32, tag="tmpb")
                            nc.vector.tensor_scalar(
                                out=tmp[:], in0=sel4[:, tt, :],
                                scalar1=a_sb[:, tsc:tsc + 1], scalar2=None,
                                op0=mybir.AluOpType.mult)
                            nc.vector.tensor_tensor(
                                out=tmp[:], in0=tmp[:], in1=beta_sb[:],
                                op=mybir.AluOpType.add)
                            nc.scalar.activation(
                                out=out_sb[:, tsc, :], in_=tmp[:],
                                func=mybir.ActivationFunctionType.Relu)

                out_view = out[e0s: e0s + 32 * P, :].rearrange(
                    "(t p) d -> p t d", p=P)
                nc.sync.dma_start(out=out_view, in_=out_sb[:])

            # ================= phase 1 (phase 2 interleaved) ==============
            s_next = 0
            for b in range(p.nblk):
                base_e = b * p.t_blk * P

                h_sb = hpool.tile([P, p.t_blk * P], bf16, tag="hblk")
                nc.sync.dma_start(out=h_sb[:],
                                  in_=h_t[:, base_e: base_e + p.t_blk * P])
                xs_sb = xspool.tile([P, p.t_blk * P], bf16, tag="xsblk")
                nc.sync.dma_start(out=xs_sb[:],
                                  in_=xs_t[:, base_e: base_e + p.t_blk * P])
                dr_sb = rpool.tile([1, p.t_blk * P], bf16, tag="drow")
                nc.sync.dma_start(out=dr_sb[:],
                                  in_=dstrel_row_in[:, base_e: base_e + p.t_blk * P])

                xt_b = hpool.tile([P, P], bf16, tag="xtb")
                nc.sync.dma_start(out=xt_b[:], in_=x_tl[:, b * P:(b + 1) * P])
                ps_xa = psC.tile([P, P], f32, tag="psxa")
                nc.tensor.matmul(out=ps_xa[:], lhsT=xt_b[:], rhs=w1aT_sb[:],
                                 start=True, stop=True)
                xa_sb = bpool.tile([P, P], bf16, tag="xasb")
                nc.scalar.copy(out=xa_sb[:], in_=ps_xa[:])

                ps_seg = psC.tile([P, P], f32, tag="seg")
                for g in range(n4):
                    t0 = g * 4
                    c0 = t0 * P
                    # broadcast dst_rel row -> [128, 512] psum
                    bc = psB.tile([P, 4 * P], f32, tag="bc")
                    nc.tensor.matmul(out=bc[:], lhsT=ones_sb[:],
                                     rhs=dr_sb[:, c0:c0 + 4 * P],
                                     start=True, stop=True)
                    # ohT[node, e] = (node == dst_rel[e])   (DVE, batched)
                    ohT = epool.tile([P, 4 * P], bf16, tag="ohT")
                    nc.vector.tensor_scalar(
                        out=ohT[:], in0=bc[:], scalar1=iotac_sb[:],
                        scalar2=None, op0=mybir.AluOpType.is_equal)
                    # oh[e, node] = (iota == dst_rel[e]) per tile  (pool)
                    oh4 = epool.tile([P, 4, P], bf16, tag="oh4")
                    for tt in range(4):
                        nc.gpsimd.tensor_scalar(
                            out=oh4[:, tt, :], in0=iotar_sb[:],
                            scalar1=dcol_sb[:, b * p.t_blk + t0 + tt:
                                            b * p.t_blk + t0 + tt + 1],
                            scalar2=None, op0=mybir.AluOpType.is_equal)
                    ps4 = psA.tile([P, 4, P], f32, tag="m4")
                    for tt in range(4):
                        nc.tensor.matmul(out=ps4[:, tt, :],
                                         lhsT=h_sb[:, c0 + tt * P: c0 + (tt + 1) * P],
                                         rhs=w1cT_sb[:], start=True, stop=False)
                        nc.tensor.matmul(out=ps4[:, tt, :],
                                         lhsT=xs_sb[:, c0 + tt * P: c0 + (tt + 1) * P],
                                         rhs=w1bT_sb[:], start=False, stop=False)
                        nc.tensor.matmul(out=ps4[:, tt, :],
                                         lhsT=ohT[:, tt * P:(tt + 1) * P],
                                         rhs=xa_sb[:], start=False, stop=True)
                    me4 = epool.tile([P, 4, P], bf16, tag="me4")
                    nc.scalar.activation(out=me4[:], in_=ps4[:],
                                         func=mybir.ActivationFunctionType.Relu)
                    for tt in range(4):
                        nc.tensor.matmul(out=ps_seg[:], lhsT=me4[:, tt, :],
                                         rhs=oh4[:, tt, :],
                                         start=(g == 0 and tt == 0),
                                         stop=(g == n4 - 1 and tt == 3))

                # ---- phase 1.5
                mnT = bpool.tile([P, P], bf16, tag="mnT")
                nc.vector.tensor_copy(out=mnT[:], in_=ps_seg[:])
                ps_y = psC.tile([P, P], f32, tag="psxa")
                nc.tensor.matmul(out=ps_y[:], lhsT=mnT[:], rhs=w2T_sb[:],
                                 start=True, stop=True)
                summ = bpool.tile([P, 1], f32, tag="summ")
                nc.scalar.activation(out=scratch[:], in_=ps_y[:],
                                     func=mybir.ActivationFunctionType.Copy,
                                     accum_out=summ[:])
                sumsq = bpool.tile([P, 1], f32, tag="sumsq")
                nc.scalar.activation(out=scratch[:], in_=ps_y[:],
                                     func=mybir.ActivationFunctionType.Square,
                                     accum_out=sumsq[:])
                negmu = bpool.tile([P, 1], f32, tag="negmu")
                nc.vector.tensor_scalar_mul(negmu[:], summ[:], -1.0 / P)
                musq = bpool.tile([P, 1], f32, tag="musq")
                nc.vector.tensor_tensor(out=musq[:], in0=summ[:], in1=summ[:],
                                        op=mybir.AluOpType.mult)
                # 128*var = sumsq - musq/128
                nc.vector.scalar_tensor_tensor(
                    out=rhs_blk[b][:, P:P + 1], in0=musq[:], scalar=-1.0 / P,
                    in1=sumsq[:], op0=mybir.AluOpType.mult,
                    op1=mybir.AluOpType.add)
                if not use_beta:
                    if use_gamma:
                        yc = bpool.tile([P, P], f32, tag="ycg")
                        nc.scalar.activation(
                            out=yc[:], in_=ps_y[:],
                            func=mybir.ActivationFunctionType.Identity,
                            bias=negmu[:])
                        nc.vector.tensor_tensor(
                            out=scratch[:], in0=yc[:], in1=gamma_sb[:],
                            op=mybir.AluOpType.mult)
                        nc.scalar.activation(
                            out=rhs_blk[b][:, 0:P], in_=scratch[:],
                            func=mybir.ActivationFunctionType.Relu)
                    else:
                        nc.scalar.activation(
                            out=rhs_blk[b][:, 0:P], in_=ps_y[:],
                            func=mybir.ActivationFunctionType.Relu,
                            bias=negmu[:])
                else:
                    yc = bpool.tile([P, P], f32, tag="ycg")
                    nc.scalar.activation(
                        out=yc[:], in_=ps_y[:],
                        func=mybir.ActivationFunctionType.Identity,
                        bias=negmu[:])
                    if use_gamma:
                        nc.vector.tensor_tensor(
                            out=rhs_blk[b][:, 0:P], in0=yc[:], in1=gamma_sb[:],
                            op=mybir.AluOpType.mult)
                    else:
                        nc.vector.tensor_copy(out=rhs_blk[b][:, 0:P], in_=yc[:])

                # interleave ready phase-2 superchunks
                while s_next < p.n_sc and \
                        min((32 * (s_next + 1) - 1) // p.t2blk, p.nblk - 1) <= b:
                    emit_superchunk(s_next)
                    s_next += 1

            while s_next < p.n_sc:
                emit_superchunk(s_next)
                s_next += 1

    nc.finalize()
    return nc


# ----------------------------------------------------------------------------
# driver
# ----------------------------------------------------------------------------


def _prep_inputs(p: Plan, x, h, snorm_n, W1, W2, ln_gamma, ln_beta):
    D = P
    use_gamma = not np.allclose(ln_gamma, 1.0)
    use_beta = not np.allclose(ln_beta, 0.0)

    x_bf = np.asarray(x).astype(BF16)

    common = {
        "w1aT": np.ascontiguousarray(W1[:, :D].T).astype(BF16),
        "w1bT": np.ascontiguousarray(W1[:, D:2 * D].T).astype(BF16),
        "w1cT": np.ascontiguousarray(W1[:, 2 * D:3 * D].T).astype(BF16),
        "w2T": np.ascontiguousarray(W2.T).astype(BF16),
        "ones_row": np.ones((1, P), dtype=BF16),
        "iota_col": np.arange(P, dtype=np.float32).reshape(P, 1),
        "iota_row": np.tile(np.arange(P, dtype=np.float32), (P, 1)).astype(BF16),
    }
    if use_gamma:
        common["gamma_b"] = np.tile(np.asarray(ln_gamma, np.float32), (P, 1))
    if use_beta:
        common["beta_b"] = np.tile(np.asarray(ln_beta, np.float32), (P, 1))

    in_maps, slots2_all = [], []
    for c in range(p.nc):
        m, slots2 = p.core_inputs(c, x_bf, h, snorm_n)
        m.update(common)
        in_maps.append(m)
        slots2_all.append(slots2)
    return in_maps, slots2_all, use_gamma, use_beta


def run(x, h, snorm_n, W1, W2, ln_gamma, ln_beta, src, dst, n_cores=8,
        trace=False):
    n_nodes, n_edges = x.shape[0], h.shape[0]
    p = Plan(n_nodes, n_edges, src, dst, nc=n_cores)
    in_maps, slots2_all, use_gamma, use_beta = _prep_inputs(
        p, x, h, snorm_n, W1, W2, ln_gamma, ln_beta)
    nc = build_program(p, use_gamma, use_beta)
    res = run_bass_kernel_spmd(nc, in_maps, core_ids=list(range(n_cores)),
                               trace=trace)
    out = np.empty((n_edges, P), dtype=np.float32)
    for c in range(n_cores):
        o = res.results[c]["out"]
        s = slots2_all[c]
        real = s >= 0
        out[s[real]] = o[real].astype(np.float32)
    return out, res


def kernel(x, h, snorm_n, snorm_e, W1, W2, ln_gamma, ln_beta, src, dst):
    out, _ = run(np.asarray(x), np.asarray(h), np.asarray(snorm_n),
                 np.asarray(W1), np.asarray(W2), np.asarray(ln_gamma),
                 np.asarray(ln_beta), np.asarray(src), np.asarray(dst))
    return out


# revision 24
# speedup vs baseline: 2.3423x; 2.3423x over previous
"""MPNN layer on 8 Trainium2 NeuronCores (Bass/Tile) - v4, gather-free.

Math (reference):
    m_edge = relu(x[dst] @ W1a^T + x[src] @ W1b^T + h @ W1c^T)        [E, D]
    m_node = segment_sum(m_edge, dst, N)                               [N, D]
    y      = m_node @ W2^T                                             [N, D]
    out_e  = relu(LN(snorm_n_e * y[src_e]))                            [E, D]

LN decomposition (exact, s>0):
    LN(s*v)*gamma+beta = (v - mu_v)*gamma * a_e + beta,
    a_e = s*rsqrt(s^2*var_v + eps) = rsqrt(var_v + eps/s^2)
and for beta==0:  relu(yc*gamma * a_e) = a_e * relu(yc*gamma)  (a_e > 0),
so the per-node record stores relu'd centered y and var; the per-edge part is
a scalar multiply.

Sharding (no collectives, no device gathers; each core independent):
  phase 1: edges bucketed by dst-owner core + 128-node dst block; h and
    x[src] are laid out per-edge by the host (input permutation / gather on
    host). Per-edge terms via per-tile matmuls; x[dst] selected on PE via a
    one-hot built from broadcast+is_equal; segment-sum via one-hot matmul.
  phase 1.5: per block: y = m@W2, mean/var/relu-center; records stay in SBUF.
  phase 2: edges bucketed by src-owner core + src block (records are local).
    One-hot select of record + var on PE; a = 1/sqrt((varx+c2x)/128) on
    column vectors; per-edge scale fused into the psum->sbuf copy. Phase-2
    superchunks are interleaved into the block loop as their blocks complete.
  Output written bf16 in bucketed order; host inverts the permutation.
"""

import numpy as np
import ml_dtypes

from concourse import bacc, tile, mybir
from concourse.bass_utils import run_bass_kernel_spmd

P = 128
LN_EPS = 1e-5
BF16 = ml_dtypes.bfloat16

# ----------------------------------------------------------------------------
# host-side planning
# ----------------------------------------------------------------------------


def _ceil_to(x, m):
    return -(-x // m) * m


class Plan:
    def __init__(self, n_nodes, n_edges, src, dst, nc=8):
        self.nc = nc
        self.n_nodes = n_nodes
        self.n_edges = n_edges
        self.npc = n_nodes // nc
        assert self.npc * nc == n_nodes
        self.npc_pad = _ceil_to(self.npc, P)
        self.nblk = self.npc_pad // P

        src = np.asarray(src).astype(np.int64)
        dst = np.asarray(dst).astype(np.int64)
        self.src, self.dst = src, dst

        # ---- phase 1: bucket edges by (dst core, dst block)
        core1 = dst // self.npc
        blk1 = (dst - core1 * self.npc) // P
        key1 = core1 * self.nblk + blk1
        t1max = 1
        self.p1 = []           # [core][block] -> ids
        for c in range(nc):
            blocks = []
            for b in range(self.nblk):
                ids = np.nonzero(key1 == c * self.nblk + b)[0]
                blocks.append(ids)
                t1max = max(t1max, -(-len(ids) // P))
            self.p1.append(blocks)
        self.t_blk = _ceil_to(t1max, 4)         # tiles per block, mult of 4
        self.t1 = self.nblk * self.t_blk
        self.e1 = self.t1 * P

        # ---- phase 2: bucket edges by (src core, src block)
        core2 = src // self.npc
        blk2 = (src - core2 * self.npc) // P
        key2 = core2 * self.nblk + blk2
        t2b = 1
        self.p2 = []
        for c in range(nc):
            blocks = []
            for b in range(self.nblk):
                ids = np.nonzero(key2 == c * self.nblk + b)[0]
                blocks.append(ids)
                t2b = max(t2b, -(-len(ids) // P))
            self.p2.append(blocks)
        self.t2blk = t2b
        self.t2 = _ceil_to(self.nblk * self.t2blk, 32)
        self.n_sc = self.t2 // 32
        self.e2 = self.t2 * P

    # ---- per-core input arrays -------------------------------------------
    def core_inputs(self, c, x_bf, h, snorm_n):
        p = self
        f32 = np.float32

        slots = np.full(p.e1, -1, dtype=np.int64)
        for b, ids in enumerate(p.p1[c]):
            base = b * p.t_blk * P
            slots[base: base + len(ids)] = ids
        pad = slots < 0
        e_ids = np.where(pad, 0, slots)

        h_t = np.ascontiguousarray(h[e_ids].T).astype(BF16)
        h_t[:, pad] = BF16(0.0)
        xs_t = np.ascontiguousarray(x_bf[self.src[e_ids]].T)
        xs_t[:, pad] = BF16(0.0)

        dst_rel = (self.dst[e_ids] - c * p.npc -
                   (np.arange(p.e1) // (p.t_blk * P)) * P)
        dst_rel = np.where(pad, -1.0, dst_rel.astype(f32)).astype(f32)
        dstrel_col = dst_rel.reshape(p.t1, P).T.copy()        # [128, t1] f32
        dstrel_row = dst_rel.astype(BF16).reshape(1, p.e1)    # [1, e1] bf16

        # phase-2 slots
        slots2 = np.full(p.e2, -1, dtype=np.int64)
        for b, ids in enumerate(p.p2[c]):
            base = b * p.t2blk * P
            slots2[base: base + len(ids)] = ids
        pad2 = slots2 < 0
        e2_ids = np.where(pad2, 0, slots2)
        src_rel = self.src[e2_ids] - c * p.npc - \
            np.minimum(np.arange(p.e2) // (p.t2blk * P), p.nblk - 1) * P
        src_rel = np.where(pad2, -1.0, src_rel.astype(f32)).astype(f32)
        srcrel_row = src_rel.astype(BF16).reshape(1, p.e2)

        s = snorm_n.reshape(-1)[e2_ids].astype(np.float64)
        with np.errstate(divide="ignore", over="ignore"):
            c2x = P * LN_EPS / (s * s)          # 128 * eps / s^2  (inf ok)
        c2x = np.where(pad2, 1.0, c2x).astype(f32)
        c2c = c2x.reshape(p.t2, P).T.copy()                   # [128, t2]

        return {
            "h_t": h_t,
            "xs_t": xs_t,
            "dstrel_col": dstrel_col,
            "dstrel_row": dstrel_row,
            "srcrel_row": srcrel_row,
            "c2c": c2c,
            "x_tl": np.ascontiguousarray(np.pad(
                x_bf[c * p.npc: (c + 1) * p.npc],
                ((0, p.npc_pad - p.npc), (0, 0))).T),
        }, slots2


# ----------------------------------------------------------------------------
# bass program
# ----------------------------------------------------------------------------


def build_program(p: Plan, use_gamma, use_beta):
    dt = mybir.dt
    nc = bacc.Bacc(None)

    f32, bf16 = dt.float32, dt.bfloat16
    REC = 132          # rhs_blk row: [relu(yc*gamma)(128) | 128*var | pad]

    x_tl = nc.declare_dram_parameter("x_tl", [P, p.npc_pad], bf16, isOutput=False)
    h_t = nc.declare_dram_parameter("h_t", [P, p.e1], bf16, isOutput=False)
    xs_t = nc.declare_dram_parameter("xs_t", [P, p.e1], bf16, isOutput=False)
    w1aT = nc.declare_dram_parameter("w1aT", [P, P], bf16, isOutput=False)
    w1bT = nc.declare_dram_parameter("w1bT", [P, P], bf16, isOutput=False)
    w1cT = nc.declare_dram_parameter("w1cT", [P, P], bf16, isOutput=False)
    w2T = nc.declare_dram_parameter("w2T", [P, P], bf16, isOutput=False)
    dstrel_col_in = nc.declare_dram_parameter("dstrel_col", [P, p.t1], f32, isOutput=False)
    dstrel_row_in = nc.declare_dram_parameter("dstrel_row", [1, p.e1], bf16, isOutput=False)
    srcrel_row_in = nc.declare_dram_parameter("srcrel_row", [1, p.e2], bf16, isOutput=False)
    c2c_in = nc.declare_dram_parameter("c2c", [P, p.t2], f32, isOutput=False)
    ones_in = nc.declare_dram_parameter("ones_row", [1, P], bf16, isOutput=False)
    iota_in = nc.declare_dram_parameter("iota_col", [P, 1], f32, isOutput=False)
    iota_row_in = nc.declare_dram_parameter("iota_row", [P, P], bf16, isOutput=False)
    gamma_b = beta_b = None
    if use_gamma:
        gamma_b = nc.declare_dram_parameter("gamma_b", [P, P], f32, isOutput=False)
    if use_beta:
        beta_b = nc.declare_dram_parameter("beta_b", [P, P], f32, isOutput=False)

    out = nc.declare_dram_parameter("out", [p.e2, P], bf16, isOutput=True)

    with tile.TileContext(nc) as tc:
        with tc.tile_pool(name="const", bufs=1) as cpool, \
             tc.tile_pool(name="hx", bufs=2) as hpool, \
             tc.tile_pool(name="xsx", bufs=2) as xspool, \
             tc.tile_pool(name="row", bufs=2) as rpool, \
             tc.tile_pool(name="edge", bufs=3) as epool, \
             tc.tile_pool(name="blk", bufs=2) as bpool, \
             tc.tile_pool(name="oh2", bufs=9) as o2pool, \
             tc.tile_pool(name="p2s", bufs=2) as s2pool, \
             tc.tile_pool(name="outp", bufs=2) as opool, \
             tc.tile_pool(name="psA", bufs=2, space="PSUM") as psA, \
             tc.tile_pool(name="psB", bufs=2, space="PSUM") as psB, \
             tc.tile_pool(name="psC", bufs=2, space="PSUM") as psC:

            # ---- constants
            w1aT_sb = cpool.tile([P, P], bf16, tag="w1a")
            w1bT_sb = cpool.tile([P, P], bf16, tag="w1b")
            w1cT_sb = cpool.tile([P, P], bf16, tag="w1c")
            w2T_sb = cpool.tile([P, P], bf16, tag="w2")
            ones_sb = cpool.tile([1, P], bf16, tag="ones")
            iotac_sb = cpool.tile([P, 1], f32, tag="iotac")
            iotar_sb = cpool.tile([P, P], bf16, tag="iotar")
            dcol_sb = cpool.tile([P, p.t1], f32, tag="dcol")
            c2c_sb = cpool.tile([P, p.t2], f32, tag="c2c")
            nc.sync.dma_start(out=w1aT_sb[:], in_=w1aT[:])
            nc.sync.dma_start(out=w1bT_sb[:], in_=w1bT[:])
            nc.sync.dma_start(out=w1cT_sb[:], in_=w1cT[:])
            nc.sync.dma_start(out=w2T_sb[:], in_=w2T[:])
            nc.sync.dma_start(out=ones_sb[:], in_=ones_in[:])
            nc.sync.dma_start(out=iotac_sb[:], in_=iota_in[:])
            nc.sync.dma_start(out=iotar_sb[:], in_=iota_row_in[:])
            nc.sync.dma_start(out=dcol_sb[:], in_=dstrel_col_in[:])
            nc.sync.dma_start(out=c2c_sb[:], in_=c2c_in[:])
            gamma_sb = beta_sb = None
            if use_gamma:
                gamma_sb = cpool.tile([P, P], f32, tag="gam")
                nc.sync.dma_start(out=gamma_sb[:], in_=gamma_b[:])
            if use_beta:
                beta_sb = cpool.tile([P, P], f32, tag="bet")
                nc.sync.dma_start(out=beta_sb[:], in_=beta_b[:])

            # per-block records, persistent
            rhs_blk = [cpool.tile([P, REC], bf16, tag=f"rec{b}",
                                  name=f"rec{b}")
                       for b in range(p.nblk)]

            scratch = cpool.tile([P, P], f32, tag="scr")
            n4 = p.t_blk // 4

            # ---------------- phase 2 superchunk emitter ------------------
            def emit_superchunk(s):
                e0s = s * 32 * P
                sr_sb = rpool.tile([1, 32 * P], bf16, tag="srow")
                nc.sync.dma_start(out=sr_sb[:],
                                  in_=srcrel_row_in[:, e0s: e0s + 32 * P])

                psVc = psC.tile([P, 32], f32, tag="psxa")
                ohTs = []
                for g in range(8):
                    c0 = g * 4 * P
                    bc2 = psB.tile([P, 4 * P], f32, tag="bc")
                    nc.tensor.matmul(out=bc2[:], lhsT=ones_sb[:],
                                     rhs=sr_sb[:, c0:c0 + 4 * P],
                                     start=True, stop=True)
                    ohT2 = o2pool.tile([P, 4 * P], bf16, tag="ohT2")
                    nc.vector.tensor_scalar(
                        out=ohT2[:], in0=bc2[:], scalar1=iotac_sb[:],
                        scalar2=None, op0=mybir.AluOpType.is_equal)
                    ohTs.append(ohT2)
                    for tt in range(4):
                        t = s * 32 + g * 4 + tt
                        b = min(t // p.t2blk, p.nblk - 1)
                        nc.tensor.matmul(
                            out=psVc[:, g * 4 + tt: g * 4 + tt + 1],
                            lhsT=ohT2[:, tt * P:(tt + 1) * P],
                            rhs=rhs_blk[b][:, P:P + 1],
                            start=True, stop=True)
                vc = s2pool.tile([P, 32], f32, tag="vc")
                nc.vector.tensor_tensor(out=vc[:], in0=psVc[:],
                                        in1=c2c_sb[:, s * 32:(s + 1) * 32],
                                        op=mybir.AluOpType.add)
                rt = s2pool.tile([P, 32], f32, tag="rt")
                nc.scalar.activation(out=rt[:], in_=vc[:],
                                     func=mybir.ActivationFunctionType.Sqrt,
                                     scale=1.0 / P)
                a_sb = s2pool.tile([P, 32], f32, tag="a_sb")
                nc.vector.reciprocal(out=a_sb[:], in_=rt[:])

                out_sb = opool.tile([P, 32, P], bf16, tag="osb")
                for g in range(8):
                    ohT2 = ohTs[g]
                    sel4 = psA.tile([P, 4, P], f32, tag="m4")
                    for tt in range(4):
                        t = s * 32 + g * 4 + tt
                        b = min(t // p.t2blk, p.nblk - 1)
                        nc.tensor.matmul(out=sel4[:, tt, :],
                                         lhsT=ohT2[:, tt * P:(tt + 1) * P],
                                         rhs=rhs_blk[b][:, 0:P],
                                         start=True, stop=True)
                    for tt in range(4):
                        tsc = g * 4 + tt
                        if not use_beta:
                            if tt % 2 == 0:
                                nc.scalar.activation(
                                    out=out_sb[:, tsc, :], in_=sel4[:, tt, :],
                                    func=mybir.ActivationFunctionType.Copy,
                                    scale=a_sb[:, tsc:tsc + 1])
                            else:
                                nc.vector.tensor_scalar(
                                    out=out_sb[:, tsc, :], in0=sel4[:, tt, :],
                                    scalar1=a_sb[:, tsc:tsc + 1], scalar2=None,
                                    op0=mybir.AluOpType.mult)
                        else:
                            tmp = s2pool.tile([P, P], f32, tag="tmpb")
                            nc.vector.tensor_scalar(
                                out=tmp[:], in0=sel4[:, tt, :],
                                scalar1=a_sb[:, tsc:tsc + 1], scalar2=None,
                                op0=mybir.AluOpType.mult)
                            nc.vector.tensor_tensor(
                                out=tmp[:], in0=tmp[:], in1=beta_sb[:],
                                op=mybir.AluOpType.add)
                            nc.scalar.activation(
                                out=out_sb[:, tsc, :], in_=tmp[:],
                                func=mybir.ActivationFunctionType.Relu)

                out_view = out[e0s: e0s + 32 * P, :].rearrange(
                    "(t p) d -> p t d", p=P)
                nc.sync.dma_start(out=out_view, in_=out_sb[:])

            # ================= phase 1 (phase 2 interleaved) ==============
            s_next = 0
            for b in range(p.nblk):
                base_e = b * p.t_blk * P

                h_sb = hpool.tile([P, p.t_blk * P], bf16, tag="hblk")
                nc.sync.dma_start(out=h_sb[:],
                                  in_=h_t[:, base_e: base_e + p.t_blk * P])
                xs_sb = xspool.tile([P, p.t_blk * P], bf16, tag="xsblk")
                nc.sync.dma_start(out=xs_sb[:],
                                  in_=xs_t[:, base_e: base_e + p.t_blk * P])
                dr_sb = rpool.tile([1, p.t_blk * P], bf16, tag="drow")
                nc.sync.dma_start(out=dr_sb[:],
                                  in_=dstrel_row_in[:, base_e: base_e + p.t_blk * P])

                xt_b = hpool.tile([P, P], bf16, tag="xtb")
                nc.sync.dma_start(out=xt_b[:], in_=x_tl[:, b * P:(b + 1) * P])
                ps_xa = psC.tile([P, P], f32, tag="psxa")
                nc.tensor.matmul(out=ps_xa[:], lhsT=xt_b[:], rhs=w1aT_sb[:],
                                 start=True, stop=True)
                xa_sb = bpool.tile([P, P], bf16, tag="xasb")
                nc.scalar.copy(out=xa_sb[:], in_=ps_xa[:])

                ps_seg = psC.tile([P, P], f32, tag="seg")
                for g in range(n4):
                    t0 = g * 4
                    c0 = t0 * P
                    # broadcast dst_rel row -> [128, 512] psum
                    bc = psB.tile([P, 4 * P], f32, tag="bc")
                    nc.tensor.matmul(out=bc[:], lhsT=ones_sb[:],
                                     rhs=dr_sb[:, c0:c0 + 4 * P],
                                     start=True, stop=True)
                    # ohT[node, e] = (node == dst_rel[e])   (DVE, batched)
                    ohT = epool.tile([P, 4 * P], bf16, tag="ohT")
                    nc.vector.tensor_scalar(
                        out=ohT[:], in0=bc[:], scalar1=iotac_sb[:],
                        scalar2=None, op0=mybir.AluOpType.is_equal)
                    # oh[e, node] = (iota == dst_rel[e]) per tile  (pool)
                    oh4 = epool.tile([P, 4, P], bf16, tag="oh4")
                    for tt in range(4):
                        nc.vector.tensor_scalar(
                            out=oh4[:, tt, :], in0=iotar_sb[:],
                            scalar1=dcol_sb[:, b * p.t_blk + t0 + tt:
                                            b * p.t_blk + t0 + tt + 1],
                            scalar2=None, op0=mybir.AluOpType.is_equal)
                    ps4 = psA.tile([P, 4, P], f32, tag="m4")
                    for tt in range(4):
                        nc.tensor.matmul(out=ps4[:, tt, :],
                                         lhsT=h_sb[:, c0 + tt * P: c0 + (tt + 1) * P],
                                         rhs=w1cT_sb[:], start=True, stop=False)
                        nc.tensor.matmul(out=ps4[:, tt, :],
                                         lhsT=xs_sb[:, c0 + tt * P: c0 + (tt + 1) * P],
                                         rhs=w1bT_sb[:], start=False, stop=False)
                        nc.tensor.matmul(out=ps4[:, tt, :],
                                         lhsT=ohT[:, tt * P:(tt + 1) * P],
                                         rhs=xa_sb[:], start=False, stop=True)
                    me4 = epool.tile([P, 4, P], bf16, tag="me4")
                    nc.scalar.activation(out=me4[:], in_=ps4[:],
                                         func=mybir.ActivationFunctionType.Relu)
                    for tt in range(4):
                        nc.tensor.matmul(out=ps_seg[:], lhsT=me4[:, tt, :],
                                         rhs=oh4[:, tt, :],
                                         start=(g == 0 and tt == 0),
                                         stop=(g == n4 - 1 and tt == 3))

                # ---- phase 1.5
                mnT = bpool.tile([P, P], bf16, tag="mnT")
                nc.vector.tensor_copy(out=mnT[:], in_=ps_seg[:])
                ps_y = psC.tile([P, P], f32, tag="psxa")
                nc.tensor.matmul(out=ps_y[:], lhsT=mnT[:], rhs=w2T_sb[:],
                                 start=True, stop=True)
                summ = bpool.tile([P, 1], f32, tag="summ")
                nc.scalar.activation(out=scratch[:], in_=ps_y[:],
                                     func=mybir.ActivationFunctionType.Copy,
                                     accum_out=summ[:])
                sumsq = bpool.tile([P, 1], f32, tag="sumsq")
                nc.scalar.activation(out=scratch[:], in_=ps_y[:],
                                     func=mybir.ActivationFunctionType.Square,
                                     accum_out=sumsq[:])
                negmu = bpool.tile([P, 1], f32, tag="negmu")
                nc.vector.tensor_scalar_mul(negmu[:], summ[:], -1.0 / P)
                musq = bpool.tile([P, 1], f32, tag="musq")
                nc.vector.tensor_tensor(out=musq[:], in0=summ[:], in1=summ[:],
                                        op=mybir.AluOpType.mult)
                # 128*var = sumsq - musq/128
                nc.vector.scalar_tensor_tensor(
                    out=rhs_blk[b][:, P:P + 1], in0=musq[:], scalar=-1.0 / P,
                    in1=sumsq[:], op0=mybir.AluOpType.mult,
                    op1=mybir.AluOpType.add)
                if not use_beta:
                    if use_gamma:
                        yc = bpool.tile([P, P], f32, tag="ycg")
                        nc.scalar.activation(
                            out=yc[:], in_=ps_y[:],
                            func=mybir.ActivationFunctionType.Identity,
                            bias=negmu[:])
                        nc.vector.tensor_tensor(
                            out=scratch[:], in0=yc[:], in1=gamma_sb[:],
                            op=mybir.AluOpType.mult)
                        nc.scalar.activation(
                            out=rhs_blk[b][:, 0:P], in_=scratch[:],
                            func=mybir.ActivationFunctionType.Relu)
                    else:
                        nc.scalar.activation(
                            out=rhs_blk[b][:, 0:P], in_=ps_y[:],
                            func=mybir.ActivationFunctionType.Relu,
                            bias=negmu[:])
                else:
                    yc = bpool.tile([P, P], f32, tag="ycg")
                    nc.scalar.activation(
                        out=yc[:], in_=ps_y[:],
                        func=mybir.ActivationFunctionType.Identity,
                        bias=negmu[:])
                    if use_gamma:
                        nc.vector.tensor_tensor(
                            out=rhs_blk[b][:, 0:P], in0=yc[:], in1=gamma_sb[:],
                            op=mybir.AluOpType.mult)
                    else:
                        nc.vector.tensor_copy(out=rhs_blk[b][:, 0:P], in_=yc[:])

                # interleave ready phase-2 superchunks
                while s_next < p.n_sc and \
                        min((32 * (s_next + 1) - 1) // p.t2blk, p.nblk - 1) <= b:
                    emit_superchunk(s_next)
                    s_next += 1

            while s_next < p.n_sc:
                emit_superchunk(s_next)
                s_next += 1

    nc.finalize()
    return nc


# ----------------------------------------------------------------------------
# driver
# ----------------------------------------------------------------------------


def _prep_inputs(p: Plan, x, h, snorm_n, W1, W2, ln_gamma, ln_beta):
    D = P
    use_gamma = not np.allclose(ln_gamma, 1.0)
    use_beta = not np.allclose(ln_beta, 0.0)

    x_bf = np.asarray(x).astype(BF16)

    common = {
        "w1aT": np.ascontiguousarray(W1[:, :D].T).astype(BF16),
        "w1bT": np.ascontiguousarray(W1[:, D:2 * D].T).astype(BF16),
        "w1cT": np.ascontiguousarray(W1[:, 2 * D:3 * D].T).astype(BF16),
        "w2T": np.ascontiguousarray(W2.T).astype(BF16),
        "ones_row": np.ones((1, P), dtype=BF16),
        "iota_col": np.arange(P, dtype=np.float32).reshape(P, 1),
        "iota_row": np.tile(np.arange(P, dtype=np.float32), (P, 1)).astype(BF16),
    }
    if use_gamma:
        common["gamma_b"] = np.tile(np.asarray(ln_gamma, np.float32), (P, 1))
    if use_beta:
        common["beta_b"] = np.tile(np.asarray(ln_beta, np.float32), (P, 1))

    in_maps, slots2_all = [], []
    for c in range(p.nc):
        m, slots2 = p.core_inputs(c, x_bf, h, snorm_n)
        m.update(common)
        in_maps.append(m)
        slots2_all.append(slots2)
    return in_maps, slots2_all, use_gamma, use_beta


def run(x, h, snorm_n, W1, W2, ln_gamma, ln_beta, src, dst, n_cores=8,
        trace=False):
    n_nodes, n_edges = x.shape[0], h.shape[0]
    p = Plan(n_nodes, n_edges, src, dst, nc=n_cores)
    in_maps, slots2_all, use_gamma, use_beta = _prep_inputs(
        p, x, h, snorm_n, W1, W2, ln_gamma, ln_beta)
    nc = build_program(p, use_gamma, use_beta)
    res = run_bass_kernel_spmd(nc, in_maps, core_ids=list(range(n_cores)),
                               trace=trace)
    out = np.empty((n_edges, P), dtype=np.float32)
    for c in range(n_cores):
        o = res.results[c]["out"]
        s = slots2_all[c]
        real = s >= 0
        out[s[real]] = o[real].astype(np.float32)
    return out, res


def kernel(x, h, snorm_n, snorm_e, W1, W2, ln_gamma, ln_beta, src, dst):
    out, _ = run(np.asarray(x), np.asarray(h), np.asarray(snorm_n),
                 np.asarray(W1), np.asarray(W2), np.asarray(ln_gamma),
                 np.asarray(ln_beta), np.asarray(src), np.asarray(dst))
    return out


# revision 25
# speedup vs baseline: 2.9643x; 1.2656x over previous
"""MPNN layer on 8 Trainium2 NeuronCores (Bass/Tile) - v6.

Math (reference):
    m_edge = relu(x[dst] @ W1a^T + x[src] @ W1b^T + h @ W1c^T)        [E, D]
    m_node = segment_sum(m_edge, dst, N)                               [N, D]
    y      = m_node @ W2^T                                             [N, D]
    out_e  = relu(LN(snorm_n_e * y[src_e]))                            [E, D]

LN decomposition (exact, s>0):
    LN(s*v)*gamma+beta = (v - mu_v)*gamma * a_e + beta,
    a_e = s*rsqrt(s^2*var_v + eps) = rsqrt(var_v + eps/s^2)
and for beta==0:  relu(yc*gamma * a_e) = a_e * relu(yc*gamma)  (a_e > 0).

Sharding (no collectives, no device gathers; each core independent):
  phase 1: edges bucketed by dst-owner core + 128-node dst block; h, x[src]
    and x[dst] are laid out per-edge by the host (input permutation/gather on
    host, transposed, bf16). Three per-tile matmuls accumulate m_edge in
    psum; segment-sum via a one-hot matmul (one-hot built on DVE).
  phase 1.5: per block: y = m@W2, mean/var/relu-center; records stay in SBUF.
  phase 2: edges bucketed by src-owner core + src block (records are local).
    One-hot select of record + var on PE; a = 1/sqrt((varx+c2x)/128) on
    column vectors; per-edge scale fused into the psum->sbuf copy. Phase-2
    superchunks are interleaved into the block loop as their blocks complete.
  Per-block tile counts are the per-block-index maxima across cores (the
  SPMD program is shared, but block b's tile count may differ from block
  b'), minimizing padding. Output written bf16 in bucketed order; host
  inverts the permutation.
"""

import numpy as np
import ml_dtypes

from concourse import bacc, tile, mybir
from concourse.bass_utils import run_bass_kernel_spmd

P = 128
LN_EPS = 1e-5
BF16 = ml_dtypes.bfloat16

# ----------------------------------------------------------------------------
# host-side planning
# ----------------------------------------------------------------------------


def _ceil_to(x, m):
    return -(-x // m) * m


class Plan:
    def __init__(self, n_nodes, n_edges, src, dst, nc=8):
        self.nc = nc
        self.n_nodes = n_nodes
        self.n_edges = n_edges
        self.npc = n_nodes // nc
        assert self.npc * nc == n_nodes
        self.npc_pad = _ceil_to(self.npc, P)
        self.nblk = self.npc_pad // P

        src = np.asarray(src).astype(np.int64)
        dst = np.asarray(dst).astype(np.int64)
        self.src, self.dst = src, dst

        # ---- phase 1: bucket edges by (dst core, dst block)
        core1 = dst // self.npc
        blk1 = (dst - core1 * self.npc) // P
        key1 = core1 * self.nblk + blk1
        self.p1 = []           # [core][block] -> ids
        cnt1 = np.zeros((nc, self.nblk), dtype=np.int64)
        for c in range(nc):
            blocks = []
            for b in range(self.nblk):
                ids = np.nonzero(key1 == c * self.nblk + b)[0]
                blocks.append(ids)
                cnt1[c, b] = len(ids)
            self.p1.append(blocks)
        # per-block-index tile count = max over cores
        self.t1b = np.maximum(1, -(-cnt1.max(axis=0) // P))   # [nblk]
        self.t1off = np.concatenate([[0], np.cumsum(self.t1b)])
        self.t1 = int(self.t1off[-1])
        self.e1 = self.t1 * P

        # ---- phase 2: bucket edges by (src core, src block)
        core2 = src // self.npc
        blk2 = (src - core2 * self.npc) // P
        key2 = core2 * self.nblk + blk2
        self.p2 = []
        cnt2 = np.zeros((nc, self.nblk), dtype=np.int64)
        for c in range(nc):
            blocks = []
            for b in range(self.nblk):
                ids = np.nonzero(key2 == c * self.nblk + b)[0]
                blocks.append(ids)
                cnt2[c, b] = len(ids)
            self.p2.append(blocks)
        self.t2b = np.maximum(1, -(-cnt2.max(axis=0) // P))
        t2off = np.concatenate([[0], np.cumsum(self.t2b)])
        self.t2 = _ceil_to(int(t2off[-1]), 32)
        self.t2off = t2off
        self.n_sc = self.t2 // 32
        self.e2 = self.t2 * P
        # block of each phase-2 tile (pad tiles -> last block)
        b2 = np.searchsorted(t2off, np.arange(self.t2), side="right") - 1
        self.b2_of = np.minimum(b2, self.nblk - 1)

    # ---- per-core input arrays -------------------------------------------
    def core_inputs(self, c, x_bf, h, snorm_n):
        p = self
        f32 = np.float32

        slots = np.full(p.e1, -1, dtype=np.int64)
        for b, ids in enumerate(p.p1[c]):
            base = int(p.t1off[b]) * P
            slots[base: base + len(ids)] = ids
        pad = slots < 0
        e_ids = np.where(pad, 0, slots)

        h_t = np.ascontiguousarray(h[e_ids].T).astype(BF16)
        h_t[:, pad] = BF16(0.0)
        xs_t = np.ascontiguousarray(x_bf[self.src[e_ids]].T)
        xs_t[:, pad] = BF16(0.0)
        xd_t = np.ascontiguousarray(x_bf[self.dst[e_ids]].T)
        xd_t[:, pad] = BF16(0.0)

        blk_of1 = np.searchsorted(p.t1off, np.arange(p.e1) // P,
                                  side="right") - 1
        dst_rel = self.dst[e_ids] - c * p.npc - blk_of1 * P
        dst_rel = np.where(pad, -1.0, dst_rel.astype(f32)).astype(f32)
        dstrel_col = dst_rel.reshape(p.t1, P).T.copy()        # [128, t1] f32

        # phase-2 slots
        slots2 = np.full(p.e2, -1, dtype=np.int64)
        for b, ids in enumerate(p.p2[c]):
            base = int(p.t2off[b]) * P
            slots2[base: base + len(ids)] = ids
        pad2 = slots2 < 0
        e2_ids = np.where(pad2, 0, slots2)
        src_rel = self.src[e2_ids] - c * p.npc - \
            self.b2_of[np.arange(p.e2) // P] * P
        src_rel = np.where(pad2, -1.0, src_rel.astype(f32)).astype(f32)
        srcrel_row = src_rel.astype(BF16).reshape(1, p.e2)

        s = snorm_n.reshape(-1)[e2_ids].astype(np.float64)
        with np.errstate(divide="ignore", over="ignore"):
            c2x = P * LN_EPS / (s * s)          # 128 * eps / s^2  (inf ok)
        c2x = np.where(pad2, 1.0, c2x).astype(f32)
        c2c = c2x.reshape(p.t2, P).T.copy()                   # [128, t2]

        return {
            "h_t": h_t,
            "xs_t": xs_t,
            "xd_t": xd_t,
            "dstrel_col": dstrel_col,
            "srcrel_row": srcrel_row,
            "c2c": c2c,
        }, slots2


# ----------------------------------------------------------------------------
# bass program
# ----------------------------------------------------------------------------


def build_program(p: Plan, use_gamma, use_beta):
    dt = mybir.dt
    nc = bacc.Bacc(None)

    f32, bf16 = dt.float32, dt.bfloat16
    REC = 132          # rhs_blk row: [relu(yc*gamma)(128) | 128*var | pad]

    h_t = nc.declare_dram_parameter("h_t", [P, p.e1], bf16, isOutput=False)
    xs_t = nc.declare_dram_parameter("xs_t", [P, p.e1], bf16, isOutput=False)
    xd_t = nc.declare_dram_parameter("xd_t", [P, p.e1], bf16, isOutput=False)
    w1aT = nc.declare_dram_parameter("w1aT", [P, P], bf16, isOutput=False)
    w1bT = nc.declare_dram_parameter("w1bT", [P, P], bf16, isOutput=False)
    w1cT = nc.declare_dram_parameter("w1cT", [P, P], bf16, isOutput=False)
    w2T = nc.declare_dram_parameter("w2T", [P, P], bf16, isOutput=False)
    dstrel_col_in = nc.declare_dram_parameter("dstrel_col", [P, p.t1], f32, isOutput=False)
    srcrel_row_in = nc.declare_dram_parameter("srcrel_row", [1, p.e2], bf16, isOutput=False)
    c2c_in = nc.declare_dram_parameter("c2c", [P, p.t2], f32, isOutput=False)
    ones_in = nc.declare_dram_parameter("ones_row", [1, P], bf16, isOutput=False)
    iota_in = nc.declare_dram_parameter("iota_col", [P, 1], f32, isOutput=False)
    iota_row_in = nc.declare_dram_parameter("iota_row", [P, P], bf16, isOutput=False)
    gamma_b = beta_b = None
    if use_gamma:
        gamma_b = nc.declare_dram_parameter("gamma_b", [P, P], f32, isOutput=False)
    if use_beta:
        beta_b = nc.declare_dram_parameter("beta_b", [P, P], f32, isOutput=False)

    out = nc.declare_dram_parameter("out", [p.e2, P], bf16, isOutput=True)

    with tile.TileContext(nc) as tc:
        with tc.tile_pool(name="const", bufs=1) as cpool, \
             tc.tile_pool(name="hx", bufs=2) as hpool, \
             tc.tile_pool(name="xsx", bufs=2) as xspool, \
             tc.tile_pool(name="xdx", bufs=2) as xdpool, \
             tc.tile_pool(name="row", bufs=2) as rpool, \
             tc.tile_pool(name="edge", bufs=3) as epool, \
             tc.tile_pool(name="blk", bufs=2) as bpool, \
             tc.tile_pool(name="oh2", bufs=9) as o2pool, \
             tc.tile_pool(name="p2s", bufs=2) as s2pool, \
             tc.tile_pool(name="outp", bufs=2) as opool, \
             tc.tile_pool(name="psA", bufs=2, space="PSUM") as psA, \
             tc.tile_pool(name="psB", bufs=2, space="PSUM") as psB, \
             tc.tile_pool(name="psC", bufs=2, space="PSUM") as psC:

            # ---- constants
            w1aT_sb = cpool.tile([P, P], bf16, tag="w1a")
            w1bT_sb = cpool.tile([P, P], bf16, tag="w1b")
            w1cT_sb = cpool.tile([P, P], bf16, tag="w1c")
            w2T_sb = cpool.tile([P, P], bf16, tag="w2")
            ones_sb = cpool.tile([1, P], bf16, tag="ones")
            iotac_sb = cpool.tile([P, 1], f32, tag="iotac")
            iotar_sb = cpool.tile([P, P], bf16, tag="iotar")
            dcol_sb = cpool.tile([P, p.t1], f32, tag="dcol")
            c2c_sb = cpool.tile([P, p.t2], f32, tag="c2c")
            nc.sync.dma_start(out=w1aT_sb[:], in_=w1aT[:])
            nc.sync.dma_start(out=w1bT_sb[:], in_=w1bT[:])
            nc.sync.dma_start(out=w1cT_sb[:], in_=w1cT[:])
            nc.sync.dma_start(out=w2T_sb[:], in_=w2T[:])
            nc.sync.dma_start(out=ones_sb[:], in_=ones_in[:])
            nc.sync.dma_start(out=iotac_sb[:], in_=iota_in[:])
            nc.sync.dma_start(out=iotar_sb[:], in_=iota_row_in[:])
            nc.sync.dma_start(out=dcol_sb[:], in_=dstrel_col_in[:])
            nc.sync.dma_start(out=c2c_sb[:], in_=c2c_in[:])
            gamma_sb = beta_sb = None
            if use_gamma:
                gamma_sb = cpool.tile([P, P], f32, tag="gam")
                nc.sync.dma_start(out=gamma_sb[:], in_=gamma_b[:])
            if use_beta:
                beta_sb = cpool.tile([P, P], f32, tag="bet")
                nc.sync.dma_start(out=beta_sb[:], in_=beta_b[:])

            rhs_blk = [cpool.tile([P, REC], bf16, tag=f"rec{b}",
                                  name=f"rec{b}")
                       for b in range(p.nblk)]

            scratch = cpool.tile([P, P], f32, tag="scr")
            scratch2 = cpool.tile([P, P], f32, tag="scr2")
            tmax = int(p.t1b.max())

            # ---------------- phase 2 superchunk emitter ------------------
            def emit_superchunk(s):
                e0s = s * 32 * P
                sr_sb = rpool.tile([1, 32 * P], bf16, tag="srow")
                nc.sync.dma_start(out=sr_sb[:],
                                  in_=srcrel_row_in[:, e0s: e0s + 32 * P])

                psVc = psC.tile([P, 32], f32, tag="psxa")
                ohTs = []
                for g in range(8):
                    c0 = g * 4 * P
                    bc2 = psB.tile([P, 4 * P], f32, tag="bc")
                    nc.tensor.matmul(out=bc2[:], lhsT=ones_sb[:],
                                     rhs=sr_sb[:, c0:c0 + 4 * P],
                                     start=True, stop=True)
                    ohT2 = o2pool.tile([P, 4 * P], bf16, tag="ohT2")
                    nc.vector.tensor_scalar(
                        out=ohT2[:], in0=bc2[:], scalar1=iotac_sb[:],
                        scalar2=None, op0=mybir.AluOpType.is_equal)
                    ohTs.append(ohT2)
                    for tt in range(4):
                        t = s * 32 + g * 4 + tt
                        b = int(p.b2_of[t])
                        nc.tensor.matmul(
                            out=psVc[:, g * 4 + tt: g * 4 + tt + 1],
                            lhsT=ohT2[:, tt * P:(tt + 1) * P],
                            rhs=rhs_blk[b][:, P:P + 1],
                            start=True, stop=True)
                vc = s2pool.tile([P, 32], f32, tag="vc")
                nc.vector.tensor_tensor(out=vc[:], in0=psVc[:],
                                        in1=c2c_sb[:, s * 32:(s + 1) * 32],
                                        op=mybir.AluOpType.add)
                rt = s2pool.tile([P, 32], f32, tag="rt")
                nc.scalar.activation(out=rt[:], in_=vc[:],
                                     func=mybir.ActivationFunctionType.Sqrt,
                                     scale=1.0 / P)
                a_sb = s2pool.tile([P, 32], f32, tag="a_sb")
                nc.vector.reciprocal(out=a_sb[:], in_=rt[:])

                out_sb = opool.tile([P, 32, P], bf16, tag="osb")
                for g in range(8):
                    ohT2 = ohTs[g]
                    sel4 = psA.tile([P, 4, P], f32, tag="m4")
                    for tt in range(4):
                        t = s * 32 + g * 4 + tt
                        b = int(p.b2_of[t])
                        nc.tensor.matmul(out=sel4[:, tt, :],
                                         lhsT=ohT2[:, tt * P:(tt + 1) * P],
                                         rhs=rhs_blk[b][:, 0:P],
                                         start=True, stop=True)
                    for tt in range(4):
                        tsc = g * 4 + tt
                        if not use_beta:
                            if tt % 2 == 0:
                                nc.scalar.activation(
                                    out=out_sb[:, tsc, :], in_=sel4[:, tt, :],
                                    func=mybir.ActivationFunctionType.Copy,
                                    scale=a_sb[:, tsc:tsc + 1])
                            else:
                                nc.vector.tensor_scalar(
                                    out=out_sb[:, tsc, :], in0=sel4[:, tt, :],
                                    scalar1=a_sb[:, tsc:tsc + 1], scalar2=None,
                                    op0=mybir.AluOpType.mult)
                        else:
                            tmp = s2pool.tile([P, P], f32, tag="tmpb")
                            nc.vector.tensor_scalar(
                                out=tmp[:], in0=sel4[:, tt, :],
                                scalar1=a_sb[:, tsc:tsc + 1], scalar2=None,
                                op0=mybir.AluOpType.mult)
                            nc.vector.tensor_tensor(
                                out=tmp[:], in0=tmp[:], in1=beta_sb[:],
                                op=mybir.AluOpType.add)
                            nc.scalar.activation(
                                out=out_sb[:, tsc, :], in_=tmp[:],
                                func=mybir.ActivationFunctionType.Relu)

                out_view = out[e0s: e0s + 32 * P, :].rearrange(
                    "(t p) d -> p t d", p=P)
                nc.sync.dma_start(out=out_view, in_=out_sb[:])

            # ================= phase 1 (phase 2 interleaved) ==============
            s_next = 0
            for b in range(p.nblk):
                t1b = int(p.t1b[b])
                base_t = int(p.t1off[b])
                base_e = base_t * P
                nbe = t1b * P

                h_sb = hpool.tile([P, tmax * P], bf16, tag="hblk")
                nc.sync.dma_start(out=h_sb[:, 0:nbe],
                                  in_=h_t[:, base_e: base_e + nbe])
                xs_sb = xspool.tile([P, tmax * P], bf16, tag="xsblk")
                nc.sync.dma_start(out=xs_sb[:, 0:nbe],
                                  in_=xs_t[:, base_e: base_e + nbe])
                xd_sb = xdpool.tile([P, tmax * P], bf16, tag="xdblk")
                nc.sync.dma_start(out=xd_sb[:, 0:nbe],
                                  in_=xd_t[:, base_e: base_e + nbe])

                ps_seg = psC.tile([P, P], f32, tag="seg")
                n4 = -(-t1b // 4)
                for g in range(n4):
                    t0 = g * 4
                    w = min(4, t1b - t0)
                    c0 = t0 * P
                    oh4 = epool.tile([P, 4, P], bf16, tag="oh4")
                    for tt in range(w):
                        nc.vector.tensor_scalar(
                            out=oh4[:, tt, :], in0=iotar_sb[:],
                            scalar1=dcol_sb[:, base_t + t0 + tt:
                                            base_t + t0 + tt + 1],
                            scalar2=None, op0=mybir.AluOpType.is_equal)
                    ps4 = psA.tile([P, 4, P], f32, tag="m4")
                    for tt in range(w):
                        sl = slice(c0 + tt * P, c0 + (tt + 1) * P)
                        nc.tensor.matmul(out=ps4[:, tt, :],
                                         lhsT=h_sb[:, sl],
                                         rhs=w1cT_sb[:], start=True, stop=False)
                        nc.tensor.matmul(out=ps4[:, tt, :],
                                         lhsT=xs_sb[:, sl],
                                         rhs=w1bT_sb[:], start=False, stop=False)
                        nc.tensor.matmul(out=ps4[:, tt, :],
                                         lhsT=xd_sb[:, sl],
                                         rhs=w1aT_sb[:], start=False, stop=True)
                    me4 = epool.tile([P, 4, P], bf16, tag="me4")
                    nc.scalar.activation(
                        out=me4[:, 0:w, :].rearrange("p a b -> p (a b)"),
                        in_=ps4[:, 0:w, :].rearrange("p a b -> p (a b)"),
                        func=mybir.ActivationFunctionType.Relu)
                    for tt in range(w):
                        nc.tensor.matmul(out=ps_seg[:], lhsT=me4[:, tt, :],
                                         rhs=oh4[:, tt, :],
                                         start=(g == 0 and tt == 0),
                                         stop=(g == n4 - 1 and tt == w - 1))

                # ---- phase 1.5
                mnT = bpool.tile([P, P], bf16, tag="mnT")
                nc.vector.tensor_copy(out=mnT[:], in_=ps_seg[:])
                ps_y = psC.tile([P, P], f32, tag="psxa")
                nc.tensor.matmul(out=ps_y[:], lhsT=mnT[:], rhs=w2T_sb[:],
                                 start=True, stop=True)
                summ = bpool.tile([P, 1], f32, tag="summ")
                nc.scalar.activation(out=scratch[:], in_=ps_y[:],
                                     func=mybir.ActivationFunctionType.Copy,
                                     accum_out=summ[:])
                sumsq = bpool.tile([P, 1], f32, tag="sumsq")
                nc.scalar.activation(out=scratch2[:], in_=ps_y[:],
                                     func=mybir.ActivationFunctionType.Square,
                                     accum_out=sumsq[:])
                negmu = bpool.tile([P, 1], f32, tag="negmu")
                nc.vector.tensor_scalar_mul(negmu[:], summ[:], -1.0 / P)
                musq = bpool.tile([P, 1], f32, tag="musq")
                nc.vector.tensor_tensor(out=musq[:], in0=summ[:], in1=summ[:],
                                        op=mybir.AluOpType.mult)
                # 128*var = sumsq - musq/128
                nc.vector.scalar_tensor_tensor(
                    out=rhs_blk[b][:, P:P + 1], in0=musq[:], scalar=-1.0 / P,
                    in1=sumsq[:], op0=mybir.AluOpType.mult,
                    op1=mybir.AluOpType.add)
                if not use_beta:
                    if use_gamma:
                        yc = bpool.tile([P, P], f32, tag="ycg")
                        nc.scalar.activation(
                            out=yc[:], in_=ps_y[:],
                            func=mybir.ActivationFunctionType.Identity,
                            bias=negmu[:])
                        nc.vector.tensor_tensor(
                            out=scratch[:], in0=yc[:], in1=gamma_sb[:],
                            op=mybir.AluOpType.mult)
                        nc.scalar.activation(
                            out=rhs_blk[b][:, 0:P], in_=scratch[:],
                            func=mybir.ActivationFunctionType.Relu)
                    else:
                        nc.scalar.activation(
                            out=rhs_blk[b][:, 0:P], in_=ps_y[:],
                            func=mybir.ActivationFunctionType.Relu,
                            bias=negmu[:])
                else:
                    yc = bpool.tile([P, P], f32, tag="ycg")
                    nc.scalar.activation(
                        out=yc[:], in_=ps_y[:],
                        func=mybir.ActivationFunctionType.Identity,
                        bias=negmu[:])
                    if use_gamma:
                        nc.vector.tensor_tensor(
                            out=rhs_blk[b][:, 0:P], in0=yc[:], in1=gamma_sb[:],
                            op=mybir.AluOpType.mult)
                    else:
                        nc.vector.tensor_copy(out=rhs_blk[b][:, 0:P], in_=yc[:])

                while s_next < p.n_sc and \
                        int(p.b2_of[min(32 * (s_next + 1) - 1, p.t2 - 1)]) <= b:
                    emit_superchunk(s_next)
                    s_next += 1

            while s_next < p.n_sc:
                emit_superchunk(s_next)
                s_next += 1

    nc.finalize()
    return nc


# ----------------------------------------------------------------------------
# driver
# ----------------------------------------------------------------------------


def _prep_inputs(p: Plan, x, h, snorm_n, W1, W2, ln_gamma, ln_beta):
    D = P
    use_gamma = not np.allclose(ln_gamma, 1.0)
    use_beta = not np.allclose(ln_beta, 0.0)

    x_bf = np.asarray(x).astype(BF16)

    common = {
        "w1aT": np.ascontiguousarray(W1[:, :D].T).astype(BF16),
        "w1bT": np.ascontiguousarray(W1[:, D:2 * D].T).astype(BF16),
        "w1cT": np.ascontiguousarray(W1[:, 2 * D:3 * D].T).astype(BF16),
        "w2T": np.ascontiguousarray(W2.T).astype(BF16),
        "ones_row": np.ones((1, P), dtype=BF16),
        "iota_col": np.arange(P, dtype=np.float32).reshape(P, 1),
        "iota_row": np.tile(np.arange(P, dtype=np.float32), (P, 1)).astype(BF16),
    }
    if use_gamma:
        common["gamma_b"] = np.tile(np.asarray(ln_gamma, np.float32), (P, 1))
    if use_beta:
        common["beta_b"] = np.tile(np.asarray(ln_beta, np.float32), (P, 1))

    in_maps, slots2_all = [], []
    for c in range(p.nc):
        m, slots2 = p.core_inputs(c, x_bf, h, snorm_n)
        m.update(common)
        in_maps.append(m)
        slots2_all.append(slots2)
    return in_maps, slots2_all, use_gamma, use_beta


def run(x, h, snorm_n, W1, W2, ln_gamma, ln_beta, src, dst, n_cores=8,
        trace=False):
    n_nodes, n_edges = x.shape[0], h.shape[0]
    p = Plan(n_nodes, n_edges, src, dst, nc=n_cores)
    in_maps, slots2_all, use_gamma, use_beta = _prep_inputs(
        p, x, h, snorm_n, W1, W2, ln_gamma, ln_beta)
    nc = build_program(p, use_gamma, use_beta)
    res = run_bass_kernel_spmd(nc, in_maps, core_ids=list(range(n_cores)),
                               trace=trace)
    out = np.empty((n_edges, P), dtype=np.float32)
    for c in range(n_cores):
        o = res.results[c]["out"]
        s = slots2_all[c]
        real = s >= 0
        out[s[real]] = o[real].astype(np.float32)
    return out, res


def kernel(x, h, snorm_n, snorm_e, W1, W2, ln_gamma, ln_beta, src, dst):
    out, _ = run(np.asarray(x), np.asarray(h), np.asarray(snorm_n),
                 np.asarray(W1), np.asarray(W2), np.asarray(ln_gamma),
                 np.asarray(ln_beta), np.asarray(src), np.asarray(dst))
    return out


# revision 26
# speedup vs baseline: 3.2204x; 1.0864x over previous
"""MPNN layer on 8 Trainium2 NeuronCores (Bass/Tile) - v6.

Math (reference):
    m_edge = relu(x[dst] @ W1a^T + x[src] @ W1b^T + h @ W1c^T)        [E, D]
    m_node = segment_sum(m_edge, dst, N)                               [N, D]
    y      = m_node @ W2^T                                             [N, D]
    out_e  = relu(LN(snorm_n_e * y[src_e]))                            [E, D]

LN decomposition (exact, s>0):
    LN(s*v)*gamma+beta = (v - mu_v)*gamma * a_e + beta,
    a_e = s*rsqrt(s^2*var_v + eps) = rsqrt(var_v + eps/s^2)
and for beta==0:  relu(yc*gamma * a_e) = a_e * relu(yc*gamma)  (a_e > 0).

Sharding (no collectives, no device gathers; each core independent):
  phase 1: edges bucketed by dst-owner core + 128-node dst block; h, x[src]
    and x[dst] are laid out per-edge by the host (input permutation/gather on
    host, transposed, bf16). Three per-tile matmuls accumulate m_edge in
    psum; segment-sum via a one-hot matmul (one-hot built on DVE).
  phase 1.5: per block: y = m@W2, mean/var/relu-center; records stay in SBUF.
  phase 2: edges bucketed by src-owner core + src block (records are local).
    One-hot select of record + var on PE; a = 1/sqrt((varx+c2x)/128) on
    column vectors; per-edge scale fused into the psum->sbuf copy. Phase-2
    superchunks are interleaved into the block loop as their blocks complete.
  Per-block tile counts are the per-block-index maxima across cores (the
  SPMD program is shared, but block b's tile count may differ from block
  b'), minimizing padding. Output written bf16 in bucketed order; host
  inverts the permutation.
"""

import numpy as np
import ml_dtypes

from concourse import bacc, tile, mybir
from concourse.bass_utils import run_bass_kernel_spmd

P = 128
LN_EPS = 1e-5
BF16 = ml_dtypes.bfloat16

# ----------------------------------------------------------------------------
# host-side planning
# ----------------------------------------------------------------------------


def _ceil_to(x, m):
    return -(-x // m) * m


class Plan:
    def __init__(self, n_nodes, n_edges, src, dst, nc=8):
        self.nc = nc
        self.n_nodes = n_nodes
        self.n_edges = n_edges
        self.npc = n_nodes // nc
        assert self.npc * nc == n_nodes
        self.npc_pad = _ceil_to(self.npc, P)
        self.nblk = self.npc_pad // P

        src = np.asarray(src).astype(np.int64)
        dst = np.asarray(dst).astype(np.int64)
        self.src, self.dst = src, dst

        # ---- phase 1: bucket edges by (dst core, dst block)
        core1 = dst // self.npc
        blk1 = (dst - core1 * self.npc) // P
        key1 = core1 * self.nblk + blk1
        self.p1 = []           # [core][block] -> ids
        cnt1 = np.zeros((nc, self.nblk), dtype=np.int64)
        for c in range(nc):
            blocks = []
            for b in range(self.nblk):
                ids = np.nonzero(key1 == c * self.nblk + b)[0]
                blocks.append(ids)
                cnt1[c, b] = len(ids)
            self.p1.append(blocks)
        # per-block-index tile count = max over cores
        self.t1b = np.maximum(1, -(-cnt1.max(axis=0) // P))   # [nblk]
        self.t1off = np.concatenate([[0], np.cumsum(self.t1b)])
        self.t1 = int(self.t1off[-1])
        self.e1 = self.t1 * P

        # ---- phase 2: bucket edges by (src core, src block)
        core2 = src // self.npc
        blk2 = (src - core2 * self.npc) // P
        key2 = core2 * self.nblk + blk2
        self.p2 = []
        cnt2 = np.zeros((nc, self.nblk), dtype=np.int64)
        for c in range(nc):
            blocks = []
            for b in range(self.nblk):
                ids = np.nonzero(key2 == c * self.nblk + b)[0]
                blocks.append(ids)
                cnt2[c, b] = len(ids)
            self.p2.append(blocks)
        self.t2b = np.maximum(1, -(-cnt2.max(axis=0) // P))
        t2off = np.concatenate([[0], np.cumsum(self.t2b)])
        self.t2 = _ceil_to(int(t2off[-1]), 32)
        self.t2off = t2off
        self.n_sc = self.t2 // 32
        self.e2 = self.t2 * P
        # block of each phase-2 tile (pad tiles -> last block)
        b2 = np.searchsorted(t2off, np.arange(self.t2), side="right") - 1
        self.b2_of = np.minimum(b2, self.nblk - 1)

    # ---- per-core input arrays -------------------------------------------
    def core_inputs(self, c, x_bf, h, snorm_n):
        p = self
        f32 = np.float32

        slots = np.full(p.e1, -1, dtype=np.int64)
        for b, ids in enumerate(p.p1[c]):
            base = int(p.t1off[b]) * P
            slots[base: base + len(ids)] = ids
        pad = slots < 0
        e_ids = np.where(pad, 0, slots)

        h_t = np.ascontiguousarray(h[e_ids].T).astype(BF16)
        h_t[:, pad] = BF16(0.0)
        xs_t = np.ascontiguousarray(x_bf[self.src[e_ids]].T)
        xs_t[:, pad] = BF16(0.0)
        xd_t = np.ascontiguousarray(x_bf[self.dst[e_ids]].T)
        xd_t[:, pad] = BF16(0.0)

        blk_of1 = np.searchsorted(p.t1off, np.arange(p.e1) // P,
                                  side="right") - 1
        dst_rel = self.dst[e_ids] - c * p.npc - blk_of1 * P
        dst_rel = np.where(pad, -1.0, dst_rel.astype(f32)).astype(f32)
        dstrel_col = dst_rel.reshape(p.t1, P).T.copy()        # [128, t1] f32

        # phase-2 slots
        slots2 = np.full(p.e2, -1, dtype=np.int64)
        for b, ids in enumerate(p.p2[c]):
            base = int(p.t2off[b]) * P
            slots2[base: base + len(ids)] = ids
        pad2 = slots2 < 0
        e2_ids = np.where(pad2, 0, slots2)
        src_rel = self.src[e2_ids] - c * p.npc - \
            self.b2_of[np.arange(p.e2) // P] * P
        src_rel = np.where(pad2, -1.0, src_rel.astype(f32)).astype(f32)
        srcrel_row = src_rel.astype(BF16).reshape(1, p.e2)

        s = snorm_n.reshape(-1)[e2_ids].astype(np.float64)
        with np.errstate(divide="ignore", over="ignore"):
            c2x = P * LN_EPS / (s * s)          # 128 * eps / s^2  (inf ok)
        c2x = np.where(pad2, 1.0, c2x).astype(f32)
        c2c = c2x.reshape(p.t2, P).T.copy()                   # [128, t2]

        return {
            "h_t": h_t,
            "xs_t": xs_t,
            "xd_t": xd_t,
            "dstrel_col": dstrel_col,
            "srcrel_row": srcrel_row,
            "c2c": c2c,
        }, slots2


# ----------------------------------------------------------------------------
# bass program
# ----------------------------------------------------------------------------


def build_program(p: Plan, use_gamma, use_beta):
    dt = mybir.dt
    nc = bacc.Bacc(None)

    f32, bf16 = dt.float32, dt.bfloat16
    REC = 132          # rhs_blk row: [relu(yc*gamma)(128) | 128*var | pad]

    h_t = nc.declare_dram_parameter("h_t", [P, p.e1], bf16, isOutput=False)
    xs_t = nc.declare_dram_parameter("xs_t", [P, p.e1], bf16, isOutput=False)
    xd_t = nc.declare_dram_parameter("xd_t", [P, p.e1], bf16, isOutput=False)
    w1aT = nc.declare_dram_parameter("w1aT", [P, P], bf16, isOutput=False)
    w1bT = nc.declare_dram_parameter("w1bT", [P, P], bf16, isOutput=False)
    w1cT = nc.declare_dram_parameter("w1cT", [P, P], bf16, isOutput=False)
    w2T = nc.declare_dram_parameter("w2T", [P, P], bf16, isOutput=False)
    dstrel_col_in = nc.declare_dram_parameter("dstrel_col", [P, p.t1], f32, isOutput=False)
    srcrel_row_in = nc.declare_dram_parameter("srcrel_row", [1, p.e2], bf16, isOutput=False)
    c2c_in = nc.declare_dram_parameter("c2c", [P, p.t2], f32, isOutput=False)
    ones_in = nc.declare_dram_parameter("ones_row", [1, P], bf16, isOutput=False)
    iota_in = nc.declare_dram_parameter("iota_col", [P, 1], f32, isOutput=False)
    iota_row_in = nc.declare_dram_parameter("iota_row", [P, P], bf16, isOutput=False)
    gamma_b = beta_b = None
    if use_gamma:
        gamma_b = nc.declare_dram_parameter("gamma_b", [P, P], f32, isOutput=False)
    if use_beta:
        beta_b = nc.declare_dram_parameter("beta_b", [P, P], f32, isOutput=False)

    out = nc.declare_dram_parameter("out", [p.e2, P], bf16, isOutput=True)

    with tile.TileContext(nc) as tc:
        with tc.tile_pool(name="const", bufs=1) as cpool, \
             tc.tile_pool(name="hx", bufs=3) as hpool, \
             tc.tile_pool(name="xsx", bufs=3) as xspool, \
             tc.tile_pool(name="xdx", bufs=3) as xdpool, \
             tc.tile_pool(name="row", bufs=2) as rpool, \
             tc.tile_pool(name="edge", bufs=3) as epool, \
             tc.tile_pool(name="blk", bufs=2) as bpool, \
             tc.tile_pool(name="oh2", bufs=9) as o2pool, \
             tc.tile_pool(name="p2s", bufs=2) as s2pool, \
             tc.tile_pool(name="outp", bufs=2) as opool, \
             tc.tile_pool(name="psA", bufs=3, space="PSUM") as psA, \
             tc.tile_pool(name="psB", bufs=2, space="PSUM") as psB, \
             tc.tile_pool(name="psC", bufs=2, space="PSUM") as psC, \
             tc.tile_pool(name="psD", bufs=1, space="PSUM") as psD:

            # ---- constants
            w1aT_sb = cpool.tile([P, P], bf16, tag="w1a")
            w1bT_sb = cpool.tile([P, P], bf16, tag="w1b")
            w1cT_sb = cpool.tile([P, P], bf16, tag="w1c")
            w2T_sb = cpool.tile([P, P], bf16, tag="w2")
            ones_sb = cpool.tile([1, P], bf16, tag="ones")
            iotac_sb = cpool.tile([P, 1], f32, tag="iotac")
            iotar_sb = cpool.tile([P, P], bf16, tag="iotar")
            dcol_sb = cpool.tile([P, p.t1], f32, tag="dcol")
            c2c_sb = cpool.tile([P, p.t2], f32, tag="c2c")
            nc.sync.dma_start(out=w1aT_sb[:], in_=w1aT[:])
            nc.sync.dma_start(out=w1bT_sb[:], in_=w1bT[:])
            nc.sync.dma_start(out=w1cT_sb[:], in_=w1cT[:])
            nc.sync.dma_start(out=w2T_sb[:], in_=w2T[:])
            nc.sync.dma_start(out=ones_sb[:], in_=ones_in[:])
            nc.sync.dma_start(out=iotac_sb[:], in_=iota_in[:])
            nc.sync.dma_start(out=iotar_sb[:], in_=iota_row_in[:])
            nc.sync.dma_start(out=dcol_sb[:], in_=dstrel_col_in[:])
            nc.sync.dma_start(out=c2c_sb[:], in_=c2c_in[:])
            gamma_sb = beta_sb = None
            if use_gamma:
                gamma_sb = cpool.tile([P, P], f32, tag="gam")
                nc.sync.dma_start(out=gamma_sb[:], in_=gamma_b[:])
            if use_beta:
                beta_sb = cpool.tile([P, P], f32, tag="bet")
                nc.sync.dma_start(out=beta_sb[:], in_=beta_b[:])

            rhs_blk = [cpool.tile([P, REC], bf16, tag=f"rec{b}",
                                  name=f"rec{b}")
                       for b in range(p.nblk)]

            scratch = cpool.tile([P, P], f32, tag="scr")
            scratch2 = cpool.tile([P, P], f32, tag="scr2")
            tmax = int(p.t1b.max())

            # ---------------- phase 2 superchunk emitter ------------------
            def emit_superchunk(s):
                e0s = s * 32 * P
                sr_sb = rpool.tile([1, 32 * P], bf16, tag="srow")
                nc.sync.dma_start(out=sr_sb[:],
                                  in_=srcrel_row_in[:, e0s: e0s + 32 * P])

                psVc = psC.tile([P, 32], f32, tag="psxa")
                ohTs = []
                for g in range(8):
                    c0 = g * 4 * P
                    bc2 = psB.tile([P, 4 * P], f32, tag="bc")
                    nc.tensor.matmul(out=bc2[:], lhsT=ones_sb[:],
                                     rhs=sr_sb[:, c0:c0 + 4 * P],
                                     start=True, stop=True)
                    ohT2 = o2pool.tile([P, 4 * P], bf16, tag="ohT2")
                    nc.vector.tensor_scalar(
                        out=ohT2[:], in0=bc2[:], scalar1=iotac_sb[:],
                        scalar2=None, op0=mybir.AluOpType.is_equal)
                    ohTs.append(ohT2)
                    for tt in range(4):
                        t = s * 32 + g * 4 + tt
                        b = int(p.b2_of[t])
                        nc.tensor.matmul(
                            out=psVc[:, g * 4 + tt: g * 4 + tt + 1],
                            lhsT=ohT2[:, tt * P:(tt + 1) * P],
                            rhs=rhs_blk[b][:, P:P + 1],
                            start=True, stop=True)
                vc = s2pool.tile([P, 32], f32, tag="vc")
                nc.vector.tensor_tensor(out=vc[:], in0=psVc[:],
                                        in1=c2c_sb[:, s * 32:(s + 1) * 32],
                                        op=mybir.AluOpType.add)
                rt = s2pool.tile([P, 32], f32, tag="rt")
                nc.scalar.activation(out=rt[:], in_=vc[:],
                                     func=mybir.ActivationFunctionType.Sqrt,
                                     scale=1.0 / P)
                a_sb = s2pool.tile([P, 32], f32, tag="a_sb")
                nc.vector.reciprocal(out=a_sb[:], in_=rt[:])

                out_sb = opool.tile([P, 32, P], bf16, tag="osb")
                for g in range(8):
                    ohT2 = ohTs[g]
                    sel4 = psA.tile([P, 4, P], f32, tag="m4")
                    for tt in range(4):
                        t = s * 32 + g * 4 + tt
                        b = int(p.b2_of[t])
                        nc.tensor.matmul(out=sel4[:, tt, :],
                                         lhsT=ohT2[:, tt * P:(tt + 1) * P],
                                         rhs=rhs_blk[b][:, 0:P],
                                         start=True, stop=True)
                    for tt in range(4):
                        tsc = g * 4 + tt
                        if not use_beta:
                            if tt % 4 != 3:
                                nc.scalar.activation(
                                    out=out_sb[:, tsc, :], in_=sel4[:, tt, :],
                                    func=mybir.ActivationFunctionType.Copy,
                                    scale=a_sb[:, tsc:tsc + 1])
                            else:
                                nc.vector.tensor_scalar(
                                    out=out_sb[:, tsc, :], in0=sel4[:, tt, :],
                                    scalar1=a_sb[:, tsc:tsc + 1], scalar2=None,
                                    op0=mybir.AluOpType.mult)
                        else:
                            tmp = s2pool.tile([P, P], f32, tag="tmpb")
                            nc.vector.tensor_scalar(
                                out=tmp[:], in0=sel4[:, tt, :],
                                scalar1=a_sb[:, tsc:tsc + 1], scalar2=None,
                                op0=mybir.AluOpType.mult)
                            nc.vector.tensor_tensor(
                                out=tmp[:], in0=tmp[:], in1=beta_sb[:],
                                op=mybir.AluOpType.add)
                            nc.scalar.activation(
                                out=out_sb[:, tsc, :], in_=tmp[:],
                                func=mybir.ActivationFunctionType.Relu)

                out_view = out[e0s: e0s + 32 * P, :].rearrange(
                    "(t p) d -> p t d", p=P)
                nc.sync.dma_start(out=out_view, in_=out_sb[:])

            # ================= phase 1 (phase 2 interleaved) ==============
            s_next = 0
            for b in range(p.nblk):
                t1b = int(p.t1b[b])
                base_t = int(p.t1off[b])
                base_e = base_t * P
                nbe = t1b * P

                h_sb = hpool.tile([P, tmax * P], bf16, tag="hblk")
                nc.sync.dma_start(out=h_sb[:, 0:nbe],
                                  in_=h_t[:, base_e: base_e + nbe])
                xs_sb = xspool.tile([P, tmax * P], bf16, tag="xsblk")
                nc.sync.dma_start(out=xs_sb[:, 0:nbe],
                                  in_=xs_t[:, base_e: base_e + nbe])
                xd_sb = xdpool.tile([P, tmax * P], bf16, tag="xdblk")
                nc.sync.dma_start(out=xd_sb[:, 0:nbe],
                                  in_=xd_t[:, base_e: base_e + nbe])

                ps_seg = psD.tile([P, P], f32, tag="seg")
                n4 = -(-t1b // 4)
                for g in range(n4):
                    t0 = g * 4
                    w = min(4, t1b - t0)
                    c0 = t0 * P
                    oh4 = epool.tile([P, 4, P], bf16, tag="oh4")
                    for tt in range(w):
                        nc.vector.tensor_scalar(
                            out=oh4[:, tt, :], in0=iotar_sb[:],
                            scalar1=dcol_sb[:, base_t + t0 + tt:
                                            base_t + t0 + tt + 1],
                            scalar2=None, op0=mybir.AluOpType.is_equal)
                    ps4 = psA.tile([P, 4, P], f32, tag="m4")
                    for tt in range(w):
                        sl = slice(c0 + tt * P, c0 + (tt + 1) * P)
                        nc.tensor.matmul(out=ps4[:, tt, :],
                                         lhsT=h_sb[:, sl],
                                         rhs=w1cT_sb[:], start=True, stop=False)
                        nc.tensor.matmul(out=ps4[:, tt, :],
                                         lhsT=xs_sb[:, sl],
                                         rhs=w1bT_sb[:], start=False, stop=False)
                        nc.tensor.matmul(out=ps4[:, tt, :],
                                         lhsT=xd_sb[:, sl],
                                         rhs=w1aT_sb[:], start=False, stop=True)
                    me4 = epool.tile([P, 4, P], bf16, tag="me4")
                    nc.scalar.activation(
                        out=me4[:, 0:w, :].rearrange("p a b -> p (a b)"),
                        in_=ps4[:, 0:w, :].rearrange("p a b -> p (a b)"),
                        func=mybir.ActivationFunctionType.Relu)
                    for tt in range(w):
                        nc.tensor.matmul(out=ps_seg[:], lhsT=me4[:, tt, :],
                                         rhs=oh4[:, tt, :],
                                         start=(g == 0 and tt == 0),
                                         stop=(g == n4 - 1 and tt == w - 1))

                # ---- phase 1.5
                mnT = bpool.tile([P, P], bf16, tag="mnT")
                nc.vector.tensor_copy(out=mnT[:], in_=ps_seg[:])
                ps_y = psC.tile([P, P], f32, tag="psxa")
                nc.tensor.matmul(out=ps_y[:], lhsT=mnT[:], rhs=w2T_sb[:],
                                 start=True, stop=True)
                summ = bpool.tile([P, 1], f32, tag="summ")
                nc.scalar.activation(out=scratch[:], in_=ps_y[:],
                                     func=mybir.ActivationFunctionType.Copy,
                                     accum_out=summ[:])
                sumsq = bpool.tile([P, 1], f32, tag="sumsq")
                nc.scalar.activation(out=scratch2[:], in_=ps_y[:],
                                     func=mybir.ActivationFunctionType.Square,
                                     accum_out=sumsq[:])
                negmu = bpool.tile([P, 1], f32, tag="negmu")
                nc.vector.tensor_scalar_mul(negmu[:], summ[:], -1.0 / P)
                musq = bpool.tile([P, 1], f32, tag="musq")
                nc.vector.tensor_tensor(out=musq[:], in0=summ[:], in1=summ[:],
                                        op=mybir.AluOpType.mult)
                # 128*var = sumsq - musq/128
                nc.vector.scalar_tensor_tensor(
                    out=rhs_blk[b][:, P:P + 1], in0=musq[:], scalar=-1.0 / P,
                    in1=sumsq[:], op0=mybir.AluOpType.mult,
                    op1=mybir.AluOpType.add)
                if not use_beta:
                    if use_gamma:
                        yc = bpool.tile([P, P], f32, tag="ycg")
                        nc.scalar.activation(
                            out=yc[:], in_=ps_y[:],
                            func=mybir.ActivationFunctionType.Identity,
                            bias=negmu[:])
                        nc.vector.tensor_tensor(
                            out=scratch[:], in0=yc[:], in1=gamma_sb[:],
                            op=mybir.AluOpType.mult)
                        nc.scalar.activation(
                            out=rhs_blk[b][:, 0:P], in_=scratch[:],
                            func=mybir.ActivationFunctionType.Relu)
                    else:
                        nc.scalar.activation(
                            out=rhs_blk[b][:, 0:P], in_=ps_y[:],
                            func=mybir.ActivationFunctionType.Relu,
                            bias=negmu[:])
                else:
                    yc = bpool.tile([P, P], f32, tag="ycg")
                    nc.scalar.activation(
                        out=yc[:], in_=ps_y[:],
                        func=mybir.ActivationFunctionType.Identity,
                        bias=negmu[:])
                    if use_gamma:
                        nc.vector.tensor_tensor(
                            out=rhs_blk[b][:, 0:P], in0=yc[:], in1=gamma_sb[:],
                            op=mybir.AluOpType.mult)
                    else:
                        nc.vector.tensor_copy(out=rhs_blk[b][:, 0:P], in_=yc[:])

                while s_next < p.n_sc and \
                        int(p.b2_of[min(32 * (s_next + 1) - 1, p.t2 - 1)]) <= b:
                    emit_superchunk(s_next)
                    s_next += 1

            while s_next < p.n_sc:
                emit_superchunk(s_next)
                s_next += 1

    nc.finalize()
    return nc


# ----------------------------------------------------------------------------
# driver
# ----------------------------------------------------------------------------


def _prep_inputs(p: Plan, x, h, snorm_n, W1, W2, ln_gamma, ln_beta):
    D = P
    use_gamma = not np.allclose(ln_gamma, 1.0)
    use_beta = not np.allclose(ln_beta, 0.0)

    x_bf = np.asarray(x).astype(BF16)

    common = {
        "w1aT": np.ascontiguousarray(W1[:, :D].T).astype(BF16),
        "w1bT": np.ascontiguousarray(W1[:, D:2 * D].T).astype(BF16),
        "w1cT": np.ascontiguousarray(W1[:, 2 * D:3 * D].T).astype(BF16),
        "w2T": np.ascontiguousarray(W2.T).astype(BF16),
        "ones_row": np.ones((1, P), dtype=BF16),
        "iota_col": np.arange(P, dtype=np.float32).reshape(P, 1),
        "iota_row": np.tile(np.arange(P, dtype=np.float32), (P, 1)).astype(BF16),
    }
    if use_gamma:
        common["gamma_b"] = np.tile(np.asarray(ln_gamma, np.float32), (P, 1))
    if use_beta:
        common["beta_b"] = np.tile(np.asarray(ln_beta, np.float32), (P, 1))

    in_maps, slots2_all = [], []
    for c in range(p.nc):
        m, slots2 = p.core_inputs(c, x_bf, h, snorm_n)
        m.update(common)
        in_maps.append(m)
        slots2_all.append(slots2)
    return in_maps, slots2_all, use_gamma, use_beta


def run(x, h, snorm_n, W1, W2, ln_gamma, ln_beta, src, dst, n_cores=8,
        trace=False):
    n_nodes, n_edges = x.shape[0], h.shape[0]
    p = Plan(n_nodes, n_edges, src, dst, nc=n_cores)
    in_maps, slots2_all, use_gamma, use_beta = _prep_inputs(
        p, x, h, snorm_n, W1, W2, ln_gamma, ln_beta)
    nc = build_program(p, use_gamma, use_beta)
    res = run_bass_kernel_spmd(nc, in_maps, core_ids=list(range(n_cores)),
                               trace=trace)
    out = np.empty((n_edges, P), dtype=np.float32)
    for c in range(n_cores):
        o = res.results[c]["out"]
        s = slots2_all[c]
        real = s >= 0
        out[s[real]] = o[real].astype(np.float32)
    return out, res


def kernel(x, h, snorm_n, snorm_e, W1, W2, ln_gamma, ln_beta, src, dst):
    out, _ = run(np.asarray(x), np.asarray(h), np.asarray(snorm_n),
                 np.asarray(W1), np.asarray(W2), np.asarray(ln_gamma),
                 np.asarray(ln_beta), np.asarray(src), np.asarray(dst))
    return out


# revision 27
# speedup vs baseline: 3.3483x; 1.0397x over previous
"""MPNN layer on 8 Trainium2 NeuronCores (Bass/Tile) - v6.

Math (reference):
    m_edge = relu(x[dst] @ W1a^T + x[src] @ W1b^T + h @ W1c^T)        [E, D]
    m_node = segment_sum(m_edge, dst, N)                               [N, D]
    y      = m_node @ W2^T                                             [N, D]
    out_e  = relu(LN(snorm_n_e * y[src_e]))                            [E, D]

LN decomposition (exact, s>0):
    LN(s*v)*gamma+beta = (v - mu_v)*gamma * a_e + beta,
    a_e = s*rsqrt(s^2*var_v + eps) = rsqrt(var_v + eps/s^2)
and for beta==0:  relu(yc*gamma * a_e) = a_e * relu(yc*gamma)  (a_e > 0).

Sharding (no collectives, no device gathers; each core independent):
  phase 1: edges bucketed by dst-owner core + 128-node dst block; h, x[src]
    and x[dst] are laid out per-edge by the host (input permutation/gather on
    host, transposed, bf16). Three per-tile matmuls accumulate m_edge in
    psum; segment-sum via a one-hot matmul (one-hot built on DVE).
  phase 1.5: per block: y = m@W2, mean/var/relu-center; records stay in SBUF.
  phase 2: edges bucketed by src-owner core + src block (records are local).
    One-hot select of record + var on PE; a = 1/sqrt((varx+c2x)/128) on
    column vectors; per-edge scale fused into the psum->sbuf copy. Phase-2
    superchunks are interleaved into the block loop as their blocks complete.
  Per-block tile counts are the per-block-index maxima across cores (the
  SPMD program is shared, but block b's tile count may differ from block
  b'), minimizing padding. Output written bf16 in bucketed order; host
  inverts the permutation.
"""

import numpy as np
import ml_dtypes

from concourse import bacc, tile, mybir
from concourse.bass_utils import run_bass_kernel_spmd

P = 128
LN_EPS = 1e-5
BF16 = ml_dtypes.bfloat16

# ----------------------------------------------------------------------------
# host-side planning
# ----------------------------------------------------------------------------


def _ceil_to(x, m):
    return -(-x // m) * m


class Plan:
    def __init__(self, n_nodes, n_edges, src, dst, nc=8):
        self.nc = nc
        self.n_nodes = n_nodes
        self.n_edges = n_edges
        self.npc = n_nodes // nc
        assert self.npc * nc == n_nodes
        self.npc_pad = _ceil_to(self.npc, P)
        self.nblk = self.npc_pad // P

        src = np.asarray(src).astype(np.int64)
        dst = np.asarray(dst).astype(np.int64)
        self.src, self.dst = src, dst

        # ---- phase 1: bucket edges by (dst core, dst block)
        core1 = dst // self.npc
        blk1 = (dst - core1 * self.npc) // P
        key1 = core1 * self.nblk + blk1
        self.p1 = []           # [core][block] -> ids
        cnt1 = np.zeros((nc, self.nblk), dtype=np.int64)
        for c in range(nc):
            blocks = []
            for b in range(self.nblk):
                ids = np.nonzero(key1 == c * self.nblk + b)[0]
                blocks.append(ids)
                cnt1[c, b] = len(ids)
            self.p1.append(blocks)
        # per-block-index tile count = max over cores
        self.t1b = np.maximum(1, -(-cnt1.max(axis=0) // P))   # [nblk]
        self.t1off = np.concatenate([[0], np.cumsum(self.t1b)])
        self.t1 = int(self.t1off[-1])
        self.e1 = self.t1 * P

        # ---- phase 2: bucket edges by (src core, src block)
        core2 = src // self.npc
        blk2 = (src - core2 * self.npc) // P
        key2 = core2 * self.nblk + blk2
        self.p2 = []
        cnt2 = np.zeros((nc, self.nblk), dtype=np.int64)
        for c in range(nc):
            blocks = []
            for b in range(self.nblk):
                ids = np.nonzero(key2 == c * self.nblk + b)[0]
                blocks.append(ids)
                cnt2[c, b] = len(ids)
            self.p2.append(blocks)
        self.t2b = np.maximum(1, -(-cnt2.max(axis=0) // P))
        t2off = np.concatenate([[0], np.cumsum(self.t2b)])
        self.t2 = _ceil_to(int(t2off[-1]), 32)
        self.t2off = t2off
        self.n_sc = self.t2 // 32
        self.e2 = self.t2 * P
        # block of each phase-2 tile (pad tiles -> last block)
        b2 = np.searchsorted(t2off, np.arange(self.t2), side="right") - 1
        self.b2_of = np.minimum(b2, self.nblk - 1)

    # ---- per-core input arrays -------------------------------------------
    def core_inputs(self, c, x_bf, h, snorm_n):
        p = self
        f32 = np.float32

        slots = np.full(p.e1, -1, dtype=np.int64)
        for b, ids in enumerate(p.p1[c]):
            base = int(p.t1off[b]) * P
            slots[base: base + len(ids)] = ids
        pad = slots < 0
        e_ids = np.where(pad, 0, slots)

        h_t = np.ascontiguousarray(h[e_ids].T).astype(BF16)
        h_t[:, pad] = BF16(0.0)
        xs_t = np.ascontiguousarray(x_bf[self.src[e_ids]].T)
        xs_t[:, pad] = BF16(0.0)
        xd_t = np.ascontiguousarray(x_bf[self.dst[e_ids]].T)
        xd_t[:, pad] = BF16(0.0)

        blk_of1 = np.searchsorted(p.t1off, np.arange(p.e1) // P,
                                  side="right") - 1
        dst_rel = self.dst[e_ids] - c * p.npc - blk_of1 * P
        dst_rel = np.where(pad, -1.0, dst_rel.astype(f32)).astype(f32)
        dstrel_col = dst_rel.reshape(p.t1, P).T.copy()        # [128, t1] f32

        # phase-2 slots
        slots2 = np.full(p.e2, -1, dtype=np.int64)
        for b, ids in enumerate(p.p2[c]):
            base = int(p.t2off[b]) * P
            slots2[base: base + len(ids)] = ids
        pad2 = slots2 < 0
        e2_ids = np.where(pad2, 0, slots2)
        src_rel = self.src[e2_ids] - c * p.npc - \
            self.b2_of[np.arange(p.e2) // P] * P
        src_rel = np.where(pad2, -1.0, src_rel.astype(f32)).astype(f32)
        srcrel_row = src_rel.astype(BF16).reshape(1, p.e2)

        s = snorm_n.reshape(-1)[e2_ids].astype(np.float64)
        with np.errstate(divide="ignore", over="ignore"):
            c2x = P * LN_EPS / (s * s)          # 128 * eps / s^2  (inf ok)
        c2x = np.where(pad2, 1.0, c2x).astype(f32)
        c2c = c2x.reshape(p.t2, P).T.copy()                   # [128, t2]

        return {
            "h_t": h_t,
            "xs_t": xs_t,
            "xd_t": xd_t,
            "dstrel_col": dstrel_col,
            "srcrel_row": srcrel_row,
            "c2c": c2c,
        }, slots2


# ----------------------------------------------------------------------------
# bass program
# ----------------------------------------------------------------------------


def build_program(p: Plan, use_gamma, use_beta):
    dt = mybir.dt
    nc = bacc.Bacc(None)

    f32, bf16 = dt.float32, dt.bfloat16
    REC = 132          # rhs_blk row: [relu(yc*gamma)(128) | 128*var | pad]

    h_t = nc.declare_dram_parameter("h_t", [P, p.e1], bf16, isOutput=False)
    xs_t = nc.declare_dram_parameter("xs_t", [P, p.e1], bf16, isOutput=False)
    xd_t = nc.declare_dram_parameter("xd_t", [P, p.e1], bf16, isOutput=False)
    w1aT = nc.declare_dram_parameter("w1aT", [P, P], bf16, isOutput=False)
    w1bT = nc.declare_dram_parameter("w1bT", [P, P], bf16, isOutput=False)
    w1cT = nc.declare_dram_parameter("w1cT", [P, P], bf16, isOutput=False)
    w2T = nc.declare_dram_parameter("w2T", [P, P], bf16, isOutput=False)
    dstrel_col_in = nc.declare_dram_parameter("dstrel_col", [P, p.t1], f32, isOutput=False)
    srcrel_row_in = nc.declare_dram_parameter("srcrel_row", [1, p.e2], bf16, isOutput=False)
    c2c_in = nc.declare_dram_parameter("c2c", [P, p.t2], f32, isOutput=False)
    ones_in = nc.declare_dram_parameter("ones_row", [1, P], bf16, isOutput=False)
    iota_in = nc.declare_dram_parameter("iota_col", [P, 1], f32, isOutput=False)
    iota_row_in = nc.declare_dram_parameter("iota_row", [P, P], bf16, isOutput=False)
    gamma_b = beta_b = None
    if use_gamma:
        gamma_b = nc.declare_dram_parameter("gamma_b", [P, P], f32, isOutput=False)
    if use_beta:
        beta_b = nc.declare_dram_parameter("beta_b", [P, P], f32, isOutput=False)

    out = nc.declare_dram_parameter("out", [p.e2, P], bf16, isOutput=True)

    with tile.TileContext(nc) as tc:
        with tc.tile_pool(name="const", bufs=1) as cpool, \
             tc.tile_pool(name="hx", bufs=3) as hpool, \
             tc.tile_pool(name="xsx", bufs=3) as xspool, \
             tc.tile_pool(name="xdx", bufs=3) as xdpool, \
             tc.tile_pool(name="row", bufs=2) as rpool, \
             tc.tile_pool(name="edge", bufs=3) as epool, \
             tc.tile_pool(name="blk", bufs=2) as bpool, \
             tc.tile_pool(name="oh2", bufs=9) as o2pool, \
             tc.tile_pool(name="p2s", bufs=2) as s2pool, \
             tc.tile_pool(name="outp", bufs=2) as opool, \
             tc.tile_pool(name="psA", bufs=3, space="PSUM") as psA, \
             tc.tile_pool(name="psB", bufs=2, space="PSUM") as psB, \
             tc.tile_pool(name="psC", bufs=2, space="PSUM") as psC, \
             tc.tile_pool(name="psD", bufs=1, space="PSUM") as psD:

            # ---- constants
            w1aT_sb = cpool.tile([P, P], bf16, tag="w1a")
            w1bT_sb = cpool.tile([P, P], bf16, tag="w1b")
            w1cT_sb = cpool.tile([P, P], bf16, tag="w1c")
            w2T_sb = cpool.tile([P, P], bf16, tag="w2")
            ones_sb = cpool.tile([1, P], bf16, tag="ones")
            iotac_sb = cpool.tile([P, 1], f32, tag="iotac")
            iotar_sb = cpool.tile([P, P], bf16, tag="iotar")
            dcol_sb = cpool.tile([P, p.t1], f32, tag="dcol")
            c2c_sb = cpool.tile([P, p.t2], f32, tag="c2c")
            nc.sync.dma_start(out=w1aT_sb[:], in_=w1aT[:])
            nc.sync.dma_start(out=w1bT_sb[:], in_=w1bT[:])
            nc.sync.dma_start(out=w1cT_sb[:], in_=w1cT[:])
            nc.sync.dma_start(out=w2T_sb[:], in_=w2T[:])
            nc.sync.dma_start(out=ones_sb[:], in_=ones_in[:])
            nc.sync.dma_start(out=iotac_sb[:], in_=iota_in[:])
            nc.sync.dma_start(out=iotar_sb[:], in_=iota_row_in[:])
            nc.sync.dma_start(out=dcol_sb[:], in_=dstrel_col_in[:])
            nc.sync.dma_start(out=c2c_sb[:], in_=c2c_in[:])
            gamma_sb = beta_sb = None
            if use_gamma:
                gamma_sb = cpool.tile([P, P], f32, tag="gam")
                nc.sync.dma_start(out=gamma_sb[:], in_=gamma_b[:])
            if use_beta:
                beta_sb = cpool.tile([P, P], f32, tag="bet")
                nc.sync.dma_start(out=beta_sb[:], in_=beta_b[:])

            rhs_blk = [cpool.tile([P, REC], bf16, tag=f"rec{b}",
                                  name=f"rec{b}")
                       for b in range(p.nblk)]

            scratch = cpool.tile([P, P], f32, tag="scr")
            scratch2 = cpool.tile([P, P], f32, tag="scr2")
            tmax = int(p.t1b.max())

            # ---------------- phase 2 superchunk emitter ------------------
            def emit_superchunk(s):
                e0s = s * 32 * P
                sr_sb = rpool.tile([1, 32 * P], bf16, tag="srow")
                nc.sync.dma_start(out=sr_sb[:],
                                  in_=srcrel_row_in[:, e0s: e0s + 32 * P])

                psVc = psC.tile([P, 32], f32, tag="psxa")
                ohTs = []
                for g in range(8):
                    c0 = g * 4 * P
                    bc2 = psB.tile([P, 4 * P], f32, tag="bc")
                    nc.tensor.matmul(out=bc2[:], lhsT=ones_sb[:],
                                     rhs=sr_sb[:, c0:c0 + 4 * P],
                                     start=True, stop=True)
                    ohT2 = o2pool.tile([P, 4 * P], bf16, tag="ohT2")
                    nc.vector.tensor_scalar(
                        out=ohT2[:], in0=bc2[:], scalar1=iotac_sb[:],
                        scalar2=None, op0=mybir.AluOpType.is_equal)
                    ohTs.append(ohT2)
                    for tt in range(4):
                        t = s * 32 + g * 4 + tt
                        b = int(p.b2_of[t])
                        nc.tensor.matmul(
                            out=psVc[:, g * 4 + tt: g * 4 + tt + 1],
                            lhsT=ohT2[:, tt * P:(tt + 1) * P],
                            rhs=rhs_blk[b][:, P:P + 1],
                            start=True, stop=True)
                vc = s2pool.tile([P, 32], f32, tag="vc")
                nc.vector.tensor_tensor(out=vc[:], in0=psVc[:],
                                        in1=c2c_sb[:, s * 32:(s + 1) * 32],
                                        op=mybir.AluOpType.add)
                rt = s2pool.tile([P, 32], f32, tag="rt")
                nc.scalar.activation(out=rt[:], in_=vc[:],
                                     func=mybir.ActivationFunctionType.Sqrt,
                                     scale=1.0 / P)
                a_sb = s2pool.tile([P, 32], f32, tag="a_sb")
                nc.vector.reciprocal(out=a_sb[:], in_=rt[:])

                out_sb = opool.tile([P, 32, P], bf16, tag="osb")
                for g in range(8):
                    ohT2 = ohTs[g]
                    sel4 = psA.tile([P, 4, P], f32, tag="m4")
                    for tt in range(4):
                        t = s * 32 + g * 4 + tt
                        b = int(p.b2_of[t])
                        nc.tensor.matmul(out=sel4[:, tt, :],
                                         lhsT=ohT2[:, tt * P:(tt + 1) * P],
                                         rhs=rhs_blk[b][:, 0:P],
                                         start=True, stop=True)
                    for tt in range(4):
                        tsc = g * 4 + tt
                        if not use_beta:
                            if tt % 2 == 0:
                                nc.scalar.activation(
                                    out=out_sb[:, tsc, :], in_=sel4[:, tt, :],
                                    func=mybir.ActivationFunctionType.Copy,
                                    scale=a_sb[:, tsc:tsc + 1])
                            else:
                                nc.vector.tensor_scalar(
                                    out=out_sb[:, tsc, :], in0=sel4[:, tt, :],
                                    scalar1=a_sb[:, tsc:tsc + 1], scalar2=None,
                                    op0=mybir.AluOpType.mult)
                        else:
                            tmp = s2pool.tile([P, P], f32, tag="tmpb")
                            nc.vector.tensor_scalar(
                                out=tmp[:], in0=sel4[:, tt, :],
                                scalar1=a_sb[:, tsc:tsc + 1], scalar2=None,
                                op0=mybir.AluOpType.mult)
                            nc.vector.tensor_tensor(
                                out=tmp[:], in0=tmp[:], in1=beta_sb[:],
                                op=mybir.AluOpType.add)
                            nc.scalar.activation(
                                out=out_sb[:, tsc, :], in_=tmp[:],
                                func=mybir.ActivationFunctionType.Relu)

                out_view = out[e0s: e0s + 32 * P, :].rearrange(
                    "(t p) d -> p t d", p=P)
                nc.sync.dma_start(out=out_view, in_=out_sb[:])

            # ================= phase 1 (phase 2 interleaved) ==============
            s_next = 0
            for b in range(p.nblk):
                t1b = int(p.t1b[b])
                base_t = int(p.t1off[b])
                base_e = base_t * P
                nbe = t1b * P

                h_sb = hpool.tile([P, tmax * P], bf16, tag="hblk")
                nc.sync.dma_start(out=h_sb[:, 0:nbe],
                                  in_=h_t[:, base_e: base_e + nbe])
                xs_sb = xspool.tile([P, tmax * P], bf16, tag="xsblk")
                nc.sync.dma_start(out=xs_sb[:, 0:nbe],
                                  in_=xs_t[:, base_e: base_e + nbe])
                xd_sb = xdpool.tile([P, tmax * P], bf16, tag="xdblk")
                nc.sync.dma_start(out=xd_sb[:, 0:nbe],
                                  in_=xd_t[:, base_e: base_e + nbe])

                ps_seg = psD.tile([P, P], f32, tag="seg")
                n4 = -(-t1b // 4)
                for g in range(n4):
                    t0 = g * 4
                    w = min(4, t1b - t0)
                    c0 = t0 * P
                    oh4 = epool.tile([P, 4, P], bf16, tag="oh4")
                    for tt in range(w):
                        nc.vector.tensor_scalar(
                            out=oh4[:, tt, :], in0=iotar_sb[:],
                            scalar1=dcol_sb[:, base_t + t0 + tt:
                                            base_t + t0 + tt + 1],
                            scalar2=None, op0=mybir.AluOpType.is_equal)
                    ps4 = psA.tile([P, 4, P], f32, tag="m4")
                    for tt in range(w):
                        sl = slice(c0 + tt * P, c0 + (tt + 1) * P)
                        nc.tensor.matmul(out=ps4[:, tt, :],
                                         lhsT=h_sb[:, sl],
                                         rhs=w1cT_sb[:], start=True, stop=False)
                        nc.tensor.matmul(out=ps4[:, tt, :],
                                         lhsT=xs_sb[:, sl],
                                         rhs=w1bT_sb[:], start=False, stop=False)
                        nc.tensor.matmul(out=ps4[:, tt, :],
                                         lhsT=xd_sb[:, sl],
                                         rhs=w1aT_sb[:], start=False, stop=True)
                    me4 = epool.tile([P, 4, P], bf16, tag="me4")
                    nc.scalar.activation(
                        out=me4[:, 0:w, :].rearrange("p a b -> p (a b)"),
                        in_=ps4[:, 0:w, :].rearrange("p a b -> p (a b)"),
                        func=mybir.ActivationFunctionType.Relu)
                    for tt in range(w):
                        nc.tensor.matmul(out=ps_seg[:], lhsT=me4[:, tt, :],
                                         rhs=oh4[:, tt, :],
                                         start=(g == 0 and tt == 0),
                                         stop=(g == n4 - 1 and tt == w - 1))

                # ---- phase 1.5
                mnT = bpool.tile([P, P], bf16, tag="mnT")
                nc.scalar.copy(out=mnT[:], in_=ps_seg[:])
                ps_y = psC.tile([P, P], f32, tag="psxa")
                nc.tensor.matmul(out=ps_y[:], lhsT=mnT[:], rhs=w2T_sb[:],
                                 start=True, stop=True)
                summ = bpool.tile([P, 1], f32, tag="summ")
                nc.scalar.activation(out=scratch[:], in_=ps_y[:],
                                     func=mybir.ActivationFunctionType.Copy,
                                     accum_out=summ[:])
                sumsq = bpool.tile([P, 1], f32, tag="sumsq")
                nc.scalar.activation(out=scratch2[:], in_=ps_y[:],
                                     func=mybir.ActivationFunctionType.Square,
                                     accum_out=sumsq[:])
                negmu = bpool.tile([P, 1], f32, tag="negmu")
                nc.vector.tensor_scalar_mul(negmu[:], summ[:], -1.0 / P)
                musq = bpool.tile([P, 1], f32, tag="musq")
                nc.vector.tensor_tensor(out=musq[:], in0=summ[:], in1=summ[:],
                                        op=mybir.AluOpType.mult)
                # 128*var = sumsq - musq/128
                nc.vector.scalar_tensor_tensor(
                    out=rhs_blk[b][:, P:P + 1], in0=musq[:], scalar=-1.0 / P,
                    in1=sumsq[:], op0=mybir.AluOpType.mult,
                    op1=mybir.AluOpType.add)
                if not use_beta:
                    if use_gamma:
                        yc = bpool.tile([P, P], f32, tag="ycg")
                        nc.scalar.activation(
                            out=yc[:], in_=ps_y[:],
                            func=mybir.ActivationFunctionType.Identity,
                            bias=negmu[:])
                        nc.vector.tensor_tensor(
                            out=scratch[:], in0=yc[:], in1=gamma_sb[:],
                            op=mybir.AluOpType.mult)
                        nc.scalar.activation(
                            out=rhs_blk[b][:, 0:P], in_=scratch[:],
                            func=mybir.ActivationFunctionType.Relu)
                    else:
                        nc.scalar.activation(
                            out=rhs_blk[b][:, 0:P], in_=ps_y[:],
                            func=mybir.ActivationFunctionType.Relu,
                            bias=negmu[:])
                else:
                    yc = bpool.tile([P, P], f32, tag="ycg")
                    nc.scalar.activation(
                        out=yc[:], in_=ps_y[:],
                        func=mybir.ActivationFunctionType.Identity,
                        bias=negmu[:])
                    if use_gamma:
                        nc.vector.tensor_tensor(
                            out=rhs_blk[b][:, 0:P], in0=yc[:], in1=gamma_sb[:],
                            op=mybir.AluOpType.mult)
                    else:
                        nc.vector.tensor_copy(out=rhs_blk[b][:, 0:P], in_=yc[:])

                while s_next < p.n_sc and \
                        int(p.b2_of[min(32 * (s_next + 1) - 1, p.t2 - 1)]) <= b:
                    emit_superchunk(s_next)
                    s_next += 1

            while s_next < p.n_sc:
                emit_superchunk(s_next)
                s_next += 1

    nc.finalize()
    return nc


# ----------------------------------------------------------------------------
# driver
# ----------------------------------------------------------------------------


def _prep_inputs(p: Plan, x, h, snorm_n, W1, W2, ln_gamma, ln_beta):
    D = P
    use_gamma = not np.allclose(ln_gamma, 1.0)
    use_beta = not np.allclose(ln_beta, 0.0)

    x_bf = np.asarray(x).astype(BF16)

    common = {
        "w1aT": np.ascontiguousarray(W1[:, :D].T).astype(BF16),
        "w1bT": np.ascontiguousarray(W1[:, D:2 * D].T).astype(BF16),
        "w1cT": np.ascontiguousarray(W1[:, 2 * D:3 * D].T).astype(BF16),
        "w2T": np.ascontiguousarray(W2.T).astype(BF16),
        "ones_row": np.ones((1, P), dtype=BF16),
        "iota_col": np.arange(P, dtype=np.float32).reshape(P, 1),
        "iota_row": np.tile(np.arange(P, dtype=np.float32), (P, 1)).astype(BF16),
    }
    if use_gamma:
        common["gamma_b"] = np.tile(np.asarray(ln_gamma, np.float32), (P, 1))
    if use_beta:
        common["beta_b"] = np.tile(np.asarray(ln_beta, np.float32), (P, 1))

    in_maps, slots2_all = [], []
    for c in range(p.nc):
        m, slots2 = p.core_inputs(c, x_bf, h, snorm_n)
        m.update(common)
        in_maps.append(m)
        slots2_all.append(slots2)
    return in_maps, slots2_all, use_gamma, use_beta


def run(x, h, snorm_n, W1, W2, ln_gamma, ln_beta, src, dst, n_cores=8,
        trace=False):
    n_nodes, n_edges = x.shape[0], h.shape[0]
    p = Plan(n_nodes, n_edges, src, dst, nc=n_cores)
    in_maps, slots2_all, use_gamma, use_beta = _prep_inputs(
        p, x, h, snorm_n, W1, W2, ln_gamma, ln_beta)
    nc = build_program(p, use_gamma, use_beta)
    res = run_bass_kernel_spmd(nc, in_maps, core_ids=list(range(n_cores)),
                               trace=trace)
    out = np.empty((n_edges, P), dtype=np.float32)
    for c in range(n_cores):
        o = res.results[c]["out"]
        s = slots2_all[c]
        real = s >= 0
        out[s[real]] = o[real].astype(np.float32)
    return out, res


def kernel(x, h, snorm_n, snorm_e, W1, W2, ln_gamma, ln_beta, src, dst):
    out, _ = run(np.asarray(x), np.asarray(h), np.asarray(snorm_n),
                 np.asarray(W1), np.asarray(W2), np.asarray(ln_gamma),
                 np.asarray(ln_beta), np.asarray(src), np.asarray(dst))
    return out
